# revision 1
# baseline (speedup 1.0000x reference)
"""MultiHeadAttention TRN2 Bass kernel (B=2, S=2048, D=1024, H=16, d=64).

Sharding: 8 cores = 2 (batch) x 4 (head groups of 4 heads).
Each core computes, for its batch b and head slice hs (256 dims):
    Q^T = (Wq[hs,:] @ x_q^T + bq)    [256, 2048]   (dh on partitions)
    K^T likewise, V = x_v @ Wv[hs,:].T + bv        [2048, 256]  (s on partitions)
    per head h (dh=64): S^T = K_h @ Q_h^T  (s_k on partitions, s_q free)
    P^T = exp(S^T / 8)   (no max subtraction: scores ~ N(0,1), exp is safe)
    [O^T ; denom] = [V_h | 1]^T @ P^T   (ones column folds the softmax
                                         denominator into the PV matmul)
    O^T = O^T * (1/denom broadcast via K=1 replicate matmul)
    y_partial = O^T.T @ Wo[:, hs].T     [2048, 1024]
Host: y[b] = sum of 4 head-group partials + bo.

All matmuls run in float32r (tf32-like, 1 cycle/row on PE). Heads are
processed in pairs whose score matmuls occupy disjoint PE row groups
(contraction dim 64 at partitions 0-63 / 64-127), so the two matmuls
stream concurrently. Q/K biases are fused into the PSUM->SBUF eviction
on the scalar engine (per-partition bias), V bias via a K=1 matmul.
"""

import numpy as np

import concourse.bass as bass
import concourse.tile as tile
import concourse.mybir as mybir
from concourse import bacc
from concourse.bass_utils import run_bass_kernel_spmd

D_MODEL = 1024
NUM_HEADS = 16
HEAD_DIM = 64
B, S = 2, 2048
N_CORES = 8
HG = 4                  # head-groups
HEADS_PER_CORE = NUM_HEADS // HG        # 4
DH = HEADS_PER_CORE * HEAD_DIM          # 256 output dims per core
KT = D_MODEL // 128                     # 8 contraction tiles
ST = S // 128                           # 16 sequence tiles
SB = S // 512                           # 4 sequence blocks of 512

F32 = mybir.dt.float32
F32R = mybir.dt.float32r
AF = mybir.ActivationFunctionType

_cached_nc = None


def build_nc():
    nc = bacc.Bacc("TRN2", target_bir_lowering=False, debug=False)

    xq_t = nc.declare_dram_parameter("xq_t", [D_MODEL, S], F32, isOutput=False)
    xk_t = nc.declare_dram_parameter("xk_t", [D_MODEL, S], F32, isOutput=False)
    xv_t = nc.declare_dram_parameter("xv_t", [D_MODEL, S], F32, isOutput=False)
    wq_t = nc.declare_dram_parameter("wq_t", [D_MODEL, DH], F32, isOutput=False)
    wk_t = nc.declare_dram_parameter("wk_t", [D_MODEL, DH], F32, isOutput=False)
    wv_t = nc.declare_dram_parameter("wv_t", [D_MODEL, DH], F32, isOutput=False)
    wo_t = nc.declare_dram_parameter("wo_t", [DH, D_MODEL], F32, isOutput=False)
    bqc = nc.declare_dram_parameter("bqc", [2, 128], F32, isOutput=False)
    bkc = nc.declare_dram_parameter("bkc", [2, 128], F32, isOutput=False)
    bv = nc.declare_dram_parameter("bv", [1, DH], F32, isOutput=False)
    y = nc.declare_dram_parameter("y", [S, D_MODEL], F32, isOutput=True)

    with tile.TileContext(nc) as tc:
        _emit(nc, tc, xq_t, xk_t, xv_t, wq_t, wk_t, wv_t, wo_t, bqc, bkc, bv, y)
    nc.compile()
    return nc


def _emit(nc, tc, xq_t, xk_t, xv_t, wq_t, wk_t, wv_t, wo_t, bqc, bkc, bv, y):
    from contextlib import ExitStack

    ctx = ExitStack()
    with ctx:
        # ---- persistent pools -------------------------------------------
        persist = ctx.enter_context(tc.tile_pool(name="persist", bufs=1))
        qt = [persist.tile([128, S], F32R, tag=f"qt{m}", name=f"qt{m}")
              for m in range(2)]
        kt_sb = [persist.tile([128, S], F32R, tag=f"kt{m}", name=f"kt{m}")
                 for m in range(2)]
        v_sb = [persist.tile([128, HEADS_PER_CORE * 65], F32R, tag=f"v{i}",
                             name=f"v{i}") for i in range(ST)]
        ot = [persist.tile([128, S], F32R, tag=f"ot{m}", name=f"ot{m}")
              for m in range(2)]
        wo_r = [persist.tile([128, D_MODEL], F32R, tag=f"wo{m}", name=f"wo{m}")
                for m in range(2)]
        ones_row = persist.tile([1, S], F32R, tag="ones")
        ones_col = persist.tile([128, HEADS_PER_CORE], F32R, tag="onesc")
        bq_c = persist.tile([128, 2], F32, tag="bqc")   # per-partition bias cols
        bk_c = persist.tile([128, 2], F32, tag="bkc")
        bv_r = persist.tile([1, DH], F32R, tag="bvr")

        # ---- constants (tmp pool closed before phase 1) ------------------
        with tc.tile_pool(name="tmp1", bufs=1) as tmp_pool:
            ones_f = tmp_pool.tile([1, S], F32, tag="onesf")
            nc.vector.memset(ones_f[:], 1.0)
            nc.vector.tensor_copy(ones_row[:], ones_f[:])
            onesc_f = tmp_pool.tile([128, HEADS_PER_CORE], F32, tag="onescf")
            nc.vector.memset(onesc_f[:], 1.0)
            nc.vector.tensor_copy(ones_col[:], onesc_f[:])

            bvf = tmp_pool.tile([1, DH], F32, tag="bvf")
            nc.sync.dma_start(bvf[:], bv[:])
            nc.vector.tensor_copy(bv_r[:], bvf[:])
            # bias columns: [2, 128] dram -> [128, 2] sbuf (one DMA each,
            # partition-major)
            nc.sync.dma_start(bq_c[:], bqc.rearrange("m p -> p m"))
            nc.sync.dma_start(bk_c[:], bkc.rearrange("m p -> p m"))
            for m in range(2):
                wof = tmp_pool.tile([128, D_MODEL], F32, tag=f"wof{m}")
                nc.sync.dma_start(wof[:], wo_t[m * 128:(m + 1) * 128, :])
                nc.vector.tensor_copy(wo_r[m][:], wof[:])

        # =============== phase 1: projections ============================
        with (
            tc.tile_pool(name="wproj", bufs=1) as wpool,
            tc.tile_pool(name="xf", bufs=9) as xf_pool,
            tc.tile_pool(name="wfp", bufs=2) as wf_pool,
            tc.tile_pool(name="xr", bufs=4) as xr_pool,
            tc.tile_pool(name="psproj", bufs=4, space="PSUM") as ps_proj,
            tc.tile_pool(name="psv", bufs=4, space="PSUM") as ps_v,
        ):
            # weights: load + round to fp32r
            w_r = {}
            for name, dram in (("q", wq_t), ("k", wk_t), ("v", wv_t)):
                tiles = []
                for k in range(KT):
                    wf = wf_pool.tile([128, DH], F32, tag="wf")
                    nc.sync.dma_start(wf[:], dram[k * 128:(k + 1) * 128, :])
                    wr = wpool.tile([128, DH], F32R, tag=f"w{name}{k}",
                                    name=f"w{name}{k}")
                    nc.vector.tensor_copy(wr[:], wf[:])
                    tiles.append(wr)
                w_r[name] = tiles

            def load_xf(dram):
                tiles = []
                for k in range(KT):
                    xf = xf_pool.tile([128, S], F32, tag="xf")
                    nc.sync.dma_start(xf[:], dram[k * 128:(k + 1) * 128, :])
                    tiles.append(xf)
                return tiles

            # ---- Q^T and K^T : out[dh 128, s 512] blocks ----
            for name, dst, bias_c in (("q", qt, bq_c), ("k", kt_sb, bk_c)):
                xf_tiles = load_xf({"q": xq_t, "k": xk_t}[name])
                for nb in range(SB):
                    pss = [ps_proj.tile([128, 512], F32, tag="pp", name="pp")
                           for _ in range(2)]
                    for k in range(KT):
                        xr = xr_pool.tile([128, 512], F32R, tag="xr")
                        nc.vector.tensor_copy(
                            xr[:], xf_tiles[k][:, nb * 512:(nb + 1) * 512])
                        for m in range(2):
                            nc.tensor.matmul(
                                pss[m][:],
                                w_r[name][k][:, m * 128:(m + 1) * 128],
                                xr[:],
                                start=(k == 0), stop=(k == KT - 1),
                            )
                    for m in range(2):
                        # eviction with fused per-partition bias on ScalarE
                        nc.scalar.activation(
                            dst[m][:, nb * 512:(nb + 1) * 512], pss[m][:],
                            AF.Identity, bias=bias_c[:, m:m + 1])

            # ---- V natural layout: out[s 128, dv 256] per s-tile ----
            xf_tiles = load_xf(xv_t)
            for ib in range(SB):        # s blocks of 512 = 4 s-tiles
                pss = [ps_v.tile([128, DH], F32, tag="pv", name="pv")
                       for _ in range(4)]
                for k in range(KT):
                    xr = xr_pool.tile([128, 512], F32R, tag="xr")
                    nc.vector.tensor_copy(
                        xr[:], xf_tiles[k][:, ib * 512:(ib + 1) * 512])
                    for i4 in range(4):
                        nc.tensor.matmul(
                            pss[i4][:],
                            xr[:, i4 * 128:(i4 + 1) * 128],
                            w_r["v"][k][:],
                            start=(k == 0), stop=False,
                        )
                for i4 in range(4):
                    i = ib * 4 + i4
                    nc.tensor.matmul(
                        pss[i4][:],
                        ones_row[0:1, i * 128:(i + 1) * 128],
                        bv_r[0:1, :],
                        start=False, stop=True,
                    )
                    for h in range(HEADS_PER_CORE):
                        nc.vector.tensor_copy(
                            v_sb[i][:, h * 65:h * 65 + 64],
                            pss[i4][:, h * 64:(h + 1) * 64])
                    vv = v_sb[i].rearrange("p (h c) -> p h c", c=65)
                    nc.vector.tensor_copy(vv[:, :, 64], ones_col[:])

        # ========== phase 2: attention + fused output projection =========
        with (
            tc.tile_pool(name="pt", bufs=3) as pt_pool,
            tc.tile_pool(name="small", bufs=4) as small_pool,
            tc.tile_pool(name="ysb", bufs=2) as y_pool,
            tc.tile_pool(name="pss", bufs=2, space="PSUM") as ps_s,
            tc.tile_pool(name="psacc", bufs=2, space="PSUM") as ps_acc,
            tc.tile_pool(name="psrep", bufs=1, space="PSUM") as ps_rep,
            tc.tile_pool(name="psy", bufs=1, space="PSUM") as ps_y,
        ):
            for qb in range(SB):
                for m in range(2):          # head pair (2m, 2m+1)
                    accs = [ps_acc.tile([65, 512], F32, tag="acc", name="acc")
                            for _ in range(2)]
                    for k in range(ST):
                        # scores for both heads of the pair: the two
                        # matmuls use disjoint PE row groups (partitions
                        # 0-63 / 64-127) and stream concurrently into two
                        # adjacent PSUM banks.
                        ss = ps_s.tile([128, 1024], F32, tag="ss")
                        for p2 in range(2):
                            po = 64 * p2
                            nc.tensor.matmul(
                                ss[:, p2 * 512:(p2 + 1) * 512],
                                kt_sb[m][po:po + 64, k * 128:(k + 1) * 128],
                                qt[m][po:po + 64, qb * 512:(qb + 1) * 512],
                                start=True, stop=True,
                            )
                        pt = pt_pool.tile([128, 1024], F32R, tag="pt")
                        nc.scalar.activation(
                            pt[:], ss[:], AF.Exp,
                            scale=1.0 / float(np.sqrt(HEAD_DIM)))
                        for p2 in range(2):
                            h = 2 * m + p2
                            nc.tensor.matmul(
                                accs[p2][:],
                                v_sb[k][:, h * 65:(h + 1) * 65],
                                pt[:, p2 * 512:(p2 + 1) * 512],
                                start=(k == 0), stop=(k == ST - 1),
                            )
                    # normalize both heads of the pair
                    for p2 in range(2):
                        po = 64 * p2
                        recip = small_pool.tile([1, 512], F32R, tag="recip")
                        with nc.allow_low_precision(reason="softmax denom"):
                            nc.vector.reciprocal(recip[:], accs[p2][64:65, :])
                        rep = ps_rep.tile([64, 512], F32, tag="rep")
                        nc.tensor.matmul(
                            rep[:], ones_row[0:1, 0:64], recip[0:1, :],
                            start=True, stop=True,
                        )
                        rep_sb = small_pool.tile([64, 512], F32, tag="repsb")
                        nc.vector.tensor_copy(rep_sb[:], rep[:])
                        nc.vector.tensor_mul(
                            ot[m][po:po + 64, qb * 512:(qb + 1) * 512],
                            accs[p2][0:64, :], rep_sb[:])

                # fused output projection for this query block
                for i4 in range(4):
                    i = qb * 4 + i4
                    ysb = y_pool.tile([128, D_MODEL], F32, tag="y")
                    for nb2 in range(2):
                        ps = ps_y.tile([128, 512], F32, tag="py")
                        for m in range(2):
                            nc.tensor.matmul(
                                ps[:],
                                ot[m][:, i * 128:(i + 1) * 128],
                                wo_r[m][:, nb2 * 512:(nb2 + 1) * 512],
                                start=(m == 0), stop=(m == 1),
                            )
                        nc.vector.tensor_copy(
                            ysb[:, nb2 * 512:(nb2 + 1) * 512], ps[:])
                    nc.sync.dma_start(y[i * 128:(i + 1) * 128, :], ysb[:])


def _get_nc():
    global _cached_nc
    if _cached_nc is None:
        _cached_nc = build_nc()
    return _cached_nc


def _make_in_maps(query, key, value, Wq, bq, Wk, bk, Wv, bv, Wo):
    """Shard + transpose on host: core c = (b, hg) with b = c // HG."""
    query = np.asarray(query, dtype=np.float32)
    key = np.asarray(key, dtype=np.float32)
    value = np.asarray(value, dtype=np.float32)
    Wq, Wk, Wv, Wo = (np.asarray(w, dtype=np.float32) for w in (Wq, Wk, Wv, Wo))
    bq, bk, bv = (np.asarray(b_, dtype=np.float32) for b_ in (bq, bk, bv))
    in_maps = []
    xq_t = [np.ascontiguousarray(query[b].T) for b in range(B)]
    xk_t = [np.ascontiguousarray(key[b].T) for b in range(B)]
    xv_t = [np.ascontiguousarray(value[b].T) for b in range(B)]
    for c in range(N_CORES):
        b, hg = divmod(c, HG)
        hs = slice(hg * DH, (hg + 1) * DH)
        in_maps.append({
            "xq_t": xq_t[b],
            "xk_t": xk_t[b],
            "xv_t": xv_t[b],
            "wq_t": np.ascontiguousarray(Wq[hs, :].T),
            "wk_t": np.ascontiguousarray(Wk[hs, :].T),
            "wv_t": np.ascontiguousarray(Wv[hs, :].T),
            "wo_t": np.ascontiguousarray(Wo[:, hs].T),
            "bqc": np.ascontiguousarray(bq[hs].reshape(2, 128)),
            "bkc": np.ascontiguousarray(bk[hs].reshape(2, 128)),
            "bv": np.ascontiguousarray(bv[hs]).reshape(1, DH),
        })
    return in_maps


def run(inputs, trace=False, **spmd_kwargs):
    nc = _get_nc()
    in_maps = _make_in_maps(
        inputs["query"], inputs["key"], inputs["value"],
        inputs["Wq"], inputs["bq"], inputs["Wk"], inputs["bk"],
        inputs["Wv"], inputs["bv"], inputs["Wo"])
    res = run_bass_kernel_spmd(
        nc, in_maps, list(range(N_CORES)), trace=trace, **spmd_kwargs)
    bo = np.asarray(inputs["bo"], dtype=np.float32)
    out = np.empty((B, S, D_MODEL), dtype=np.float32)
    for b in range(B):
        acc = np.zeros((S, D_MODEL), dtype=np.float32)
        for hg in range(HG):
            acc += res.results[b * HG + hg]["y"]
        out[b] = acc + bo
    return out, res


def kernel(**inputs) -> np.ndarray:
    out, _ = run(inputs, trace=False)
    return out



# revision 22
# speedup vs baseline: 1.1380x; 1.1380x over previous
"""MultiHeadAttention TRN2 Bass kernel (B=2, S=2048, D=1024, H=16, d=64).

Sharding: 8 cores = 2 (batch) x 4 (head groups of 4 heads).
Each core computes, for its batch b and head slice hs (256 dims):
    K^T = (Wk[hs,:] @ x_k^T + bk)    [256, 2048]   (dh on partitions)
    Q^T likewise; V = x_v @ Wv[hs,:].T + bv        [2048, 256]  (s on partitions)
    per head pair (2m, 2m+1): S^T = K_h @ Q_h^T, the two heads' score
    matmuls occupy disjoint PE row groups (contraction 64 at partitions
    0-63 / 64-127) and different PSUM banks -> they stream CONCURRENTLY.
    P^T = exp(S^T / 8)   (scores ~ N(0,1), exp is safe without max-sub)
    [O^T ; denom] = [V_h | 1]^T @ P^T   (ones column folds the softmax
                                         denominator into the PV matmul)
    O^T = O^T * (1/denom)  (reciprocal_approx_fast on DVE, broadcast via
                            K=1 col-tiled replicate matmuls)
    y_partial = O^T.T @ Wo[:, hs].T     [2048, 1024], DMA'd from PSUM
Host: y[b] = sum of 4 head-group partials + bo.

All tensors are bf16 on the host side (halves DMA, removes all vector-
engine casts, keeps every matmul at 1 cycle/row). Schedule is built to
keep both the Tensor engine (HAM clock gate: sustained activity = 2.4
GHz) and the Scalar engine (exp over 16.8M score elements, the second
hard floor) continuously busy: attention on query-block 0 starts right
after the K projection + first Q block, with the V projection emitted
as PE filler between score matmuls (its PV consumption is deferred).
"""

import numpy as np
import ml_dtypes

import concourse.bass as bass
import concourse.tile as tile
import concourse.mybir as mybir
from concourse import bacc
from concourse.bass_utils import run_bass_kernel_spmd

D_MODEL = 1024
NUM_HEADS = 16
HEAD_DIM = 64
B, S = 2, 2048
N_CORES = 8
HG = 4                  # head-groups
HEADS_PER_CORE = NUM_HEADS // HG        # 4
DH = HEADS_PER_CORE * HEAD_DIM          # 256 output dims per core
KT = D_MODEL // 128                     # 8 contraction tiles
ST = S // 128                           # 16 sequence tiles
SB = S // 512                           # 4 sequence blocks of 512

F32 = mybir.dt.float32
F32R = mybir.dt.float32r
BF16 = mybir.dt.bfloat16
AF = mybir.ActivationFunctionType
BF16_NP = ml_dtypes.bfloat16

_cached_nc = None


DEBUG_DUMP = False


def build_nc():
    nc = bacc.Bacc("TRN2", target_bir_lowering=False, debug=False)

    xq_t = nc.declare_dram_parameter("xq_t", [D_MODEL, S], BF16, isOutput=False)
    xk_t = nc.declare_dram_parameter("xk_t", [D_MODEL, S], BF16, isOutput=False)
    xv_t = nc.declare_dram_parameter("xv_t", [D_MODEL, S], BF16, isOutput=False)
    wq_t = nc.declare_dram_parameter("wq_t", [D_MODEL, DH], BF16, isOutput=False)
    wk_t = nc.declare_dram_parameter("wk_t", [D_MODEL, DH], BF16, isOutput=False)
    wv_t = nc.declare_dram_parameter("wv_t", [D_MODEL, DH], BF16, isOutput=False)
    wo_t = nc.declare_dram_parameter("wo_t", [DH, D_MODEL], BF16, isOutput=False)
    bqc = nc.declare_dram_parameter("bqc", [2, 128], F32, isOutput=False)
    bkc = nc.declare_dram_parameter("bkc", [2, 128], F32, isOutput=False)
    bv = nc.declare_dram_parameter("bv", [1, DH], BF16, isOutput=False)
    y = nc.declare_dram_parameter("y", [S, D_MODEL], F32, isOutput=True)
    dbg = None
    if DEBUG_DUMP:
        dbg = {
            "qt_d": nc.declare_dram_parameter("qt_d", [256, S], F32, isOutput=True),
            "kt_d": nc.declare_dram_parameter("kt_d", [256, S], F32, isOutput=True),
            "v_d": nc.declare_dram_parameter("v_d", [S, 260], F32, isOutput=True),
            "ot_d": nc.declare_dram_parameter("ot_d", [256, S], F32, isOutput=True),
        }

    with tile.TileContext(nc) as tc:
        _emit(nc, tc, xq_t, xk_t, xv_t, wq_t, wk_t, wv_t, wo_t, bqc, bkc, bv, y,
              dbg)
    nc.compile()
    return nc


def _emit(nc, tc, xq_t, xk_t, xv_t, wq_t, wk_t, wv_t, wo_t, bqc, bkc, bv, y,
          dbg=None):
    from contextlib import ExitStack

    ctx = ExitStack()
    with ctx:
        # ---- persistent pools -------------------------------------------
        persist = ctx.enter_context(tc.tile_pool(name="persist", bufs=1))
        qt = [persist.tile([128, S], BF16, tag=f"qt{m}", name=f"qt{m}")
              for m in range(2)]
        kt_sb = [persist.tile([128, S], BF16, tag=f"kt{m}", name=f"kt{m}")
                 for m in range(2)]
        v_sb = [persist.tile([128, HEADS_PER_CORE * 65], BF16, tag=f"v{i}",
                             name=f"v{i}") for i in range(ST)]
        ot = [persist.tile([128, S], BF16, tag=f"ot{m}", name=f"ot{m}")
              for m in range(2)]
        wo_r = [persist.tile([128, D_MODEL], BF16, tag=f"wo{m}", name=f"wo{m}")
                for m in range(2)]
        ones_row = persist.tile([1, S], BF16, tag="ones")
        ones64 = persist.tile([1, 64], F32, tag="ones64")
        bq_c = persist.tile([128, 2], F32, tag="bqc")   # per-partition bias cols
        bk_c = persist.tile([128, 2], F32, tag="bkc")
        bv_r = persist.tile([1, DH], BF16, tag="bvr")

        w_sb = {}
        for name, dram in (("k", wk_t), ("q", wq_t), ("v", wv_t)):
            w_sb[name] = [persist.tile([128, DH], BF16, tag=f"w{name}{k}",
                                       name=f"w{name}{k}")
                          for k in range(KT)]

        # ---- DMA: weights first, then xk, xq, xv ------------------------
        for name in ("k", "q", "v"):
            dram = {"k": wk_t, "q": wq_t, "v": wv_t}[name]
            for k in range(KT):
                nc.sync.dma_start(w_sb[name][k][:],
                                  dram[k * 128:(k + 1) * 128, :])
        for m in range(2):
            nc.sync.dma_start(wo_r[m][:], wo_t[m * 128:(m + 1) * 128, :])
        nc.sync.dma_start(bv_r[:], bv[:])
        nc.sync.dma_start(bq_c[:], bqc.rearrange("m p -> p m"))
        nc.sync.dma_start(bk_c[:], bkc.rearrange("m p -> p m"))

        xk_pool = ctx.enter_context(tc.tile_pool(name="xk", bufs=1))
        xq_pool = ctx.enter_context(tc.tile_pool(name="xq", bufs=1))
        xv_pool = ctx.enter_context(tc.tile_pool(name="xv", bufs=1))
        xk_tiles, xq_tiles, xv_tiles = [], [], []
        for k in range(KT):
            t = xk_pool.tile([128, S], BF16, tag=f"xk{k}", name=f"xk{k}")
            nc.sync.dma_start(t[:], xk_t[k * 128:(k + 1) * 128, :])
            xk_tiles.append(t)
        for k in range(KT):
            t = xq_pool.tile([128, S], BF16, tag=f"xq{k}", name=f"xq{k}")
            nc.sync.dma_start(t[:], xq_t[k * 128:(k + 1) * 128, :])
            xq_tiles.append(t)
        for k in range(KT):
            t = xv_pool.tile([128, S], BF16, tag=f"xv{k}", name=f"xv{k}")
            nc.sync.dma_start(t[:], xv_t[k * 128:(k + 1) * 128, :])
            xv_tiles.append(t)

        # ---- pools for the pipelined body -------------------------------
        ps_s = ctx.enter_context(
            tc.tile_pool(name="pss", bufs=2, space="PSUM"))      # 4 banks
        ps_acc = ctx.enter_context(
            tc.tile_pool(name="psacc", bufs=1, space="PSUM"))    # 2 banks
        ps_w = ctx.enter_context(
            tc.tile_pool(name="psw", bufs=2, space="PSUM"))      # 2 banks
        pt_pool = ctx.enter_context(tc.tile_pool(name="pt", bufs=16))
        sm_pool = ctx.enter_context(tc.tile_pool(name="small", bufs=1))
        y_pool = ctx.enter_context(tc.tile_pool(name="ysb", bufs=2))

        # constants
        const_pool = ctx.enter_context(tc.tile_pool(name="const", bufs=1))
        ones_f = const_pool.tile([1, 512], F32, tag="onesf")
        nc.vector.memset(ones_f[:], 1.0)
        for _c in range(4):
            nc.vector.tensor_copy(ones_row[0:1, _c * 512:(_c + 1) * 512],
                                  ones_f[:])
        nc.vector.memset(ones64[:], 1.0)
        ones64_r = persist.tile([1, 64], F32R, tag="ones64r")
        nc.vector.tensor_copy(ones64_r[:], ones64[:])
        ones_col = persist.tile([128, HEADS_PER_CORE], F32, tag="onesc")
        nc.vector.memset(ones_col[:], 1.0)

        def proj_qk(name, dst, bias_c, x_tiles, nb):
            """Project one 512-col seq block of Q^T or K^T (both m halves)."""
            for m in range(2):
                ps = ps_w.tile([128, 512], F32, tag="pw", name="pw")
                for k in range(KT):
                    nc.tensor.matmul(
                        ps[:],
                        w_sb[name][k][:, m * 128:(m + 1) * 128],
                        x_tiles[k][:, nb * 512:(nb + 1) * 512],
                        start=(k == 0), stop=(k == KT - 1),
                    )
                nc.scalar.activation(
                    dst[m][:, nb * 512:(nb + 1) * 512], ps[:],
                    AF.Identity, bias=bias_c[:, m:m + 1])

        def v_chunk(i):
            """Project V for s-tile i (0..15) into v_sb. One accumulation
            group per PSUM tile: interleaved groups sharing a bank clobber
            each other's has_written state."""
            ps = ps_w.tile([128, 512], F32, tag="pw", name="pw")
            for k in range(KT):
                nc.tensor.matmul(
                    ps[:, 0:256],
                    xv_tiles[k][:, i * 128:(i + 1) * 128],
                    w_sb["v"][k][:],
                    start=(k == 0), stop=False,
                )
            nc.tensor.matmul(
                ps[:, 0:256],
                ones_row[0:1, i * 128:(i + 1) * 128],
                bv_r[0:1, :],
                start=False, stop=True,
            )
            src = ps[:, 0:256].rearrange("p (h c) -> p h c", c=64)
            vv = v_sb[i].rearrange("p (h c) -> p h c", c=65)
            nc.vector.tensor_copy(vv[:, :, 0:64], src)
            nc.vector.tensor_copy(vv[:, :, 64], ones_col[:])

        def scores(qb, m, k):
            """Score pair (heads 2m, 2m+1) for sk-tile k, sq-block qb.
            Two K=64 matmuls on disjoint PE row groups + PSUM banks run
            concurrently. Returns the bf16 exp'd tile."""
            ss = ps_s.tile([128, 1024], F32, tag="ss")
            for p2 in range(2):
                po = 64 * p2
                nc.tensor.matmul(
                    ss[:, p2 * 512:(p2 + 1) * 512],
                    kt_sb[m][po:po + 64, k * 128:(k + 1) * 128],
                    qt[m][po:po + 64, qb * 512:(qb + 1) * 512],
                    start=True, stop=True,
                )
            pt = pt_pool.tile([128, 1024], BF16, tag="pt")
            nc.scalar.activation(
                pt[:], ss[:], AF.Exp, scale=1.0 / float(np.sqrt(HEAD_DIM)))
            return pt

        def pv(m, k, pt, accs):
            for p2 in range(2):
                h = 2 * m + p2
                nc.tensor.matmul(
                    accs[p2][:],
                    v_sb[k][:, h * 65:(h + 1) * 65],
                    pt[:, p2 * 512:(p2 + 1) * 512],
                    start=(k == 0), stop=(k == ST - 1),
                )

        def norm(qb, m, accs):
            """ot[m][:, qb block] = O^T / denom for both heads of pair m."""
            for p2 in range(2):
                recip_r = sm_pool.tile([1, 512], F32R, tag="recipr")
                with nc.allow_low_precision(reason="softmax denom"):
                    nc.vector.reciprocal(recip_r[:], accs[p2][64:65, :])
                # replicate 1/denom across 64 partitions via a K=1 matmul
                rep = ps_w.tile([128, 512], F32, tag="pw", name="pw")
                nc.tensor.matmul(
                    rep[0:64, :], ones64_r[0:1, :], recip_r[0:1, :],
                    start=True, stop=True,
                )
                rep_sb = sm_pool.tile([64, 512], F32, tag="repsb")
                nc.vector.tensor_copy(rep_sb[:], rep[0:64, :])
                po = 64 * p2
                nc.vector.tensor_mul(
                    ot[m][po:po + 64, qb * 512:(qb + 1) * 512],
                    accs[p2][0:64, :], rep_sb[:])

        def yproj(qb):
            for i4 in range(4):
                i = qb * 4 + i4
                ysb = y_pool.tile([128, D_MODEL], F32, tag="ysb")
                for nb2 in range(2):
                    ps = ps_w.tile([128, 512], F32, tag="pw", name="pw")
                    for m in range(2):
                        nc.tensor.matmul(
                            ps[:],
                            ot[m][:, i * 128:(i + 1) * 128],
                            wo_r[m][:, nb2 * 512:(nb2 + 1) * 512],
                            start=(m == 0), stop=(m == 1),
                        )
                    nc.vector.tensor_copy(
                        ysb[:, nb2 * 512:(nb2 + 1) * 512], ps[:])
                nc.sync.dma_start(y[i * 128:(i + 1) * 128, :], ysb[:])

        # =============== emission schedule ===============================
        # lead-in: K fully, Q block 0  (scalar engine: evictions only)
        for nb in range(SB):
            proj_qk("k", kt_sb, bk_c, xk_tiles, nb)
        proj_qk("q", qt, bq_c, xq_tiles, 0)

        # (qb0, m0): scores with V-projection as PE filler; PV deferred
        # until V is resident.
        pts = []
        for k in range(ST):
            pts.append(scores(0, 0, k))
            v_chunk(k)
        accs = [ps_acc.tile([65, 512], F32, tag=f"acc{p}", name=f"acc{p}")
                for p in range(2)]
        for k in range(ST):
            pv(0, k, pts[k], accs)
        pts = None
        norm(0, 0, accs)

        # remaining pairs: tight scores/exp/PV ping-pong; Q projection for
        # the next block and the output projection ride along as filler.
        for qb in range(SB):
            for m in range(2):
                if qb == 0 and m == 0:
                    continue
                accs = [ps_acc.tile([65, 512], F32, tag=f"acc{p}",
                                    name=f"acc{p}") for p in range(2)]
                for k in range(ST):
                    pt = scores(qb, m, k)
                    pv(m, k, pt, accs)
                norm(qb, m, accs)
            if qb + 1 < SB:
                proj_qk("q", qt, bq_c, xq_tiles, qb + 1)
            yproj(qb)

        if dbg is not None:
            dump_pool = ctx.enter_context(tc.tile_pool(name="dump", bufs=1))
            for nm, tiles in (("qt_d", qt), ("kt_d", kt_sb), ("ot_d", ot)):
                for m in range(2):
                    df = dump_pool.tile([128, S], F32, tag="df")
                    nc.vector.tensor_copy(df[:], tiles[m][:])
                    nc.sync.dma_start(dbg[nm][m * 128:(m + 1) * 128, :], df[:])
            for i in range(ST):
                dv = dump_pool.tile([128, 260], F32, tag="dv")
                nc.vector.tensor_copy(dv[:], v_sb[i][:])
                nc.sync.dma_start(dbg["v_d"][i * 128:(i + 1) * 128, :], dv[:])


def _get_nc():
    global _cached_nc
    if _cached_nc is None:
        _cached_nc = build_nc()
    return _cached_nc


def _make_in_maps(query, key, value, Wq, bq, Wk, bk, Wv, bv, Wo):
    """Shard + transpose + bf16-cast on host: core c = (b, hg), b = c // HG."""
    query = np.asarray(query, dtype=np.float32)
    key = np.asarray(key, dtype=np.float32)
    value = np.asarray(value, dtype=np.float32)
    Wq, Wk, Wv, Wo = (np.asarray(w, dtype=np.float32) for w in (Wq, Wk, Wv, Wo))
    bq, bk, bv = (np.asarray(b_, dtype=np.float32) for b_ in (bq, bk, bv))
    in_maps = []
    xq_t = [np.ascontiguousarray(query[b].T).astype(BF16_NP) for b in range(B)]
    xk_t = [np.ascontiguousarray(key[b].T).astype(BF16_NP) for b in range(B)]
    xv_t = [np.ascontiguousarray(value[b].T).astype(BF16_NP) for b in range(B)]
    for c in range(N_CORES):
        b, hg = divmod(c, HG)
        hs = slice(hg * DH, (hg + 1) * DH)
        in_maps.append({
            "xq_t": xq_t[b],
            "xk_t": xk_t[b],
            "xv_t": xv_t[b],
            "wq_t": np.ascontiguousarray(Wq[hs, :].T).astype(BF16_NP),
            "wk_t": np.ascontiguousarray(Wk[hs, :].T).astype(BF16_NP),
            "wv_t": np.ascontiguousarray(Wv[hs, :].T).astype(BF16_NP),
            "wo_t": np.ascontiguousarray(Wo[:, hs].T).astype(BF16_NP),
            "bqc": np.ascontiguousarray(bq[hs].reshape(2, 128)),
            "bkc": np.ascontiguousarray(bk[hs].reshape(2, 128)),
            "bv": np.ascontiguousarray(bv[hs]).reshape(1, DH).astype(BF16_NP),
        })
    return in_maps


def run(inputs, trace=False, **spmd_kwargs):
    nc = _get_nc()
    in_maps = _make_in_maps(
        inputs["query"], inputs["key"], inputs["value"],
        inputs["Wq"], inputs["bq"], inputs["Wk"], inputs["bk"],
        inputs["Wv"], inputs["bv"], inputs["Wo"])
    res = run_bass_kernel_spmd(
        nc, in_maps, list(range(N_CORES)), trace=trace, **spmd_kwargs)
    bo = np.asarray(inputs["bo"], dtype=np.float32)
    out = np.empty((B, S, D_MODEL), dtype=np.float32)
    for b in range(B):
        acc = np.zeros((S, D_MODEL), dtype=np.float32)
        for hg in range(HG):
            acc += res.results[b * HG + hg]["y"]
        out[b] = acc + bo
    return out, res


def kernel(**inputs) -> np.ndarray:
    out, _ = run(inputs, trace=False)
    return out


# revision 27
# speedup vs baseline: 1.3333x; 1.1716x over previous
"""MultiHeadAttention TRN2 Bass kernel (B=2, S=2048, D=1024, H=16, d=64).

Sharding: 8 cores = 2 (batch) x 4 (head groups of 4 heads).
Each core computes, for its batch b and head slice hs (256 dims):
    K^T = (Wk[hs,:] @ x_k^T + bk)    [256, 2048]   (dh on partitions)
    Q^T likewise; V = x_v @ Wv[hs,:].T + bv        [2048, 256]  (s on partitions)
    per head pair (2m, 2m+1): S^T = K_h @ Q_h^T, the two heads' score
    matmuls occupy disjoint PE row groups (contraction 64 at partitions
    0-63 / 64-127) and different PSUM banks -> they stream concurrently.
    P^T = exp(S^T / 8)   (scores ~ N(0,1), exp is safe without max-sub)
    [O^T ; denom] = [V_h | 1]^T @ P^T   (ones column folds the softmax
                                         denominator into the PV matmul)
    O^T = O^T * (1/denom)  (reciprocal_approx_fast, PE K=1 replicate)
    y_partial = O^T.T @ Wo[:, hs].T     [2048, 1024]
Host: y[b] = sum of 4 head-group partials + bo.

Everything the PE streams is bf16 (host-side cast: halves DMA, removes
all DVE casts, 1 cycle/row matmuls). The schedule is paced by the two
hard floors: Scalar-engine exp over 16.8M score elements (~137us) and
PE matmul rows (~110us). x is DMA'd in [128,512] column chunks through
rotating pools so the first score matmul lands ~10us in; after that the
emission keeps Scalar saturated: per sk-tile the PE emits scores(k)
BEFORE pv(k-1) (software pipeline, so the PE never blocks on the exp it
is feeding), and projection/output-projection work is woven into the
per-pair PE slack.
"""

import numpy as np
import ml_dtypes

import concourse.bass as bass
import concourse.tile as tile
import concourse.mybir as mybir
from concourse import bacc
from concourse.bass_utils import run_bass_kernel_spmd

D_MODEL = 1024
NUM_HEADS = 16
HEAD_DIM = 64
B, S = 2, 2048
N_CORES = 8
HG = 4                  # head-groups
HEADS_PER_CORE = NUM_HEADS // HG        # 4
DH = HEADS_PER_CORE * HEAD_DIM          # 256 output dims per core
KT = D_MODEL // 128                     # 8 contraction tiles
ST = S // 128                           # 16 sequence tiles
SB = S // 512                           # 4 sequence blocks of 512

F32 = mybir.dt.float32
F32R = mybir.dt.float32r
BF16 = mybir.dt.bfloat16
AF = mybir.ActivationFunctionType
BF16_NP = ml_dtypes.bfloat16

_cached_nc = None


def build_nc():
    nc = bacc.Bacc("TRN2", target_bir_lowering=False, debug=False)

    xq_t = nc.declare_dram_parameter("xq_t", [D_MODEL, S], BF16, isOutput=False)
    xk_t = nc.declare_dram_parameter("xk_t", [D_MODEL, S], BF16, isOutput=False)
    xv_t = nc.declare_dram_parameter("xv_t", [D_MODEL, S], BF16, isOutput=False)
    wq_t = nc.declare_dram_parameter("wq_t", [D_MODEL, DH], BF16, isOutput=False)
    wk_t = nc.declare_dram_parameter("wk_t", [D_MODEL, DH], BF16, isOutput=False)
    wv_t = nc.declare_dram_parameter("wv_t", [D_MODEL, DH], BF16, isOutput=False)
    wo_t = nc.declare_dram_parameter("wo_t", [DH, D_MODEL], BF16, isOutput=False)
    bqc = nc.declare_dram_parameter("bqc", [2, 128], F32, isOutput=False)
    bkc = nc.declare_dram_parameter("bkc", [2, 128], F32, isOutput=False)
    bv = nc.declare_dram_parameter("bv", [1, DH], BF16, isOutput=False)
    y = nc.declare_dram_parameter("y", [S, D_MODEL], F32, isOutput=True)

    with tile.TileContext(nc) as tc:
        _emit(nc, tc, xq_t, xk_t, xv_t, wq_t, wk_t, wv_t, wo_t, bqc, bkc, bv, y)
    nc.compile()
    return nc


def _emit(nc, tc, xq_t, xk_t, xv_t, wq_t, wk_t, wv_t, wo_t, bqc, bkc, bv, y):
    from contextlib import ExitStack

    ctx = ExitStack()
    with ctx:
        # ---- persistent tiles -------------------------------------------
        persist = ctx.enter_context(tc.tile_pool(name="persist", bufs=1))
        qt = [persist.tile([128, S], BF16, tag=f"qt{m}", name=f"qt{m}")
              for m in range(2)]
        kt_sb = [persist.tile([128, S], BF16, tag=f"kt{m}", name=f"kt{m}")
                 for m in range(2)]
        v_sb = [persist.tile([128, HEADS_PER_CORE * 65], BF16, tag=f"v{i}",
                             name=f"v{i}") for i in range(ST)]
        ot = [persist.tile([128, S], BF16, tag=f"ot{m}", name=f"ot{m}")
              for m in range(2)]
        wo_r = [persist.tile([128, D_MODEL], BF16, tag=f"wo{m}", name=f"wo{m}")
                for m in range(2)]
        ones_row = persist.tile([1, S], BF16, tag="ones")
        ones64 = persist.tile([1, 64], F32, tag="ones64")
        ones64_r = persist.tile([1, 64], F32R, tag="ones64r")
        ones_col = persist.tile([128, HEADS_PER_CORE], F32, tag="onesc")
        bq_c = persist.tile([128, 2], F32, tag="bqc")   # per-partition bias
        bk_c = persist.tile([128, 2], F32, tag="bkc")
        bv_r = persist.tile([1, DH], BF16, tag="bvr")
        w_sb = {}
        for name in ("k", "q", "v"):
            w_sb[name] = [persist.tile([128, DH], BF16, tag=f"w{name}{k}",
                                       name=f"w{name}{k}")
                          for k in range(KT)]

        # ---- streaming x chunk pools ([128,512] bf16 per (k, nb)) -------
        xk_pool = ctx.enter_context(tc.tile_pool(name="xk", bufs=16))
        xq_pool = ctx.enter_context(tc.tile_pool(name="xq", bufs=16))
        xv_pool = ctx.enter_context(tc.tile_pool(name="xv", bufs=16))
        xk_c, xq_c, xv_c = {}, {}, {}

        def dma_x(pool, store, dram, nb):
            for k in range(KT):
                t = pool.tile([128, 512], BF16, tag="xc", name="xc")
                nc.sync.dma_start(
                    t[:], dram[k * 128:(k + 1) * 128,
                               nb * 512:(nb + 1) * 512])
                store[(k, nb)] = t

        # DMA priority order == consumption order
        nc.sync.dma_start(bq_c[:], bqc.rearrange("m p -> p m"))
        nc.sync.dma_start(bk_c[:], bkc.rearrange("m p -> p m"))
        for k in range(KT):
            nc.sync.dma_start(w_sb["k"][k][:], wk_t[k * 128:(k + 1) * 128, :])
        for k in range(KT):
            nc.sync.dma_start(w_sb["q"][k][:], wq_t[k * 128:(k + 1) * 128, :])
        dma_x(xk_pool, xk_c, xk_t, 0)
        dma_x(xq_pool, xq_c, xq_t, 0)
        for nb in (1, 2, 3):
            dma_x(xk_pool, xk_c, xk_t, nb)
        dma_x(xq_pool, xq_c, xq_t, 1)
        for k in range(KT):
            nc.sync.dma_start(w_sb["v"][k][:], wv_t[k * 128:(k + 1) * 128, :])
        nc.sync.dma_start(bv_r[:], bv[:])
        for nb in range(SB):
            dma_x(xv_pool, xv_c, xv_t, nb)
        dma_x(xq_pool, xq_c, xq_t, 2)
        dma_x(xq_pool, xq_c, xq_t, 3)
        for m in range(2):
            nc.sync.dma_start(wo_r[m][:], wo_t[m * 128:(m + 1) * 128, :])

        # ---- pipelined-body pools ---------------------------------------
        ps_s = ctx.enter_context(
            tc.tile_pool(name="pss", bufs=2, space="PSUM"))      # 4 banks
        ps_acc = ctx.enter_context(
            tc.tile_pool(name="psacc", bufs=1, space="PSUM"))    # 2 banks
        ps_w = ctx.enter_context(
            tc.tile_pool(name="psw", bufs=2, space="PSUM"))      # 2 banks
        pt_pool = ctx.enter_context(tc.tile_pool(name="pt", bufs=32))
        sm_pool = ctx.enter_context(tc.tile_pool(name="small", bufs=2))
        y_pool = ctx.enter_context(tc.tile_pool(name="ysb", bufs=2))

        # constants
        const_pool = ctx.enter_context(tc.tile_pool(name="const", bufs=1))
        ones_f = const_pool.tile([1, 512], F32, tag="onesf", name="onesf")
        nc.vector.memset(ones_f[:], 1.0)
        for c in range(4):
            nc.vector.tensor_copy(ones_row[0:1, c * 512:(c + 1) * 512],
                                  ones_f[:])
        nc.vector.memset(ones64[:], 1.0)
        nc.vector.tensor_copy(ones64_r[:], ones64[:])
        nc.vector.memset(ones_col[:], 1.0)

        # ---- building blocks --------------------------------------------
        def proj_qk_m(name, dst, bias_c, xc, nb, m):
            """Project one (512-col, m-half) block of Q^T or K^T."""
            ps = ps_w.tile([128, 512], F32, tag="pw", name="pw")
            for k in range(KT):
                nc.tensor.matmul(
                    ps[:],
                    w_sb[name][k][:, m * 128:(m + 1) * 128],
                    xc[(k, nb)][:],
                    start=(k == 0), stop=(k == KT - 1),
                )
            nc.scalar.activation(
                dst[m][:, nb * 512:(nb + 1) * 512], ps[:],
                AF.Identity, bias=bias_c[:, m:m + 1])

        def v_chunk(i):
            """Project V for s-tile i into v_sb[i] (+ ones column). One
            accumulation group per PSUM tile: interleaved groups sharing a
            bank clobber each other's has_written state."""
            nb, col = divmod(i, 4)
            ps = ps_w.tile([128, 512], F32, tag="pw", name="pw")
            for k in range(KT):
                nc.tensor.matmul(
                    ps[:, 0:256],
                    xv_c[(k, nb)][:, col * 128:(col + 1) * 128],
                    w_sb["v"][k][:],
                    start=(k == 0), stop=False,
                )
            nc.tensor.matmul(
                ps[:, 0:256],
                ones_row[0:1, i * 128:(i + 1) * 128],
                bv_r[0:1, :],
                start=False, stop=True,
            )
            src = ps[:, 0:256].rearrange("p (h c) -> p h c", c=64)
            vv = v_sb[i].rearrange("p (h c) -> p h c", c=65)
            nc.vector.tensor_copy(vv[:, :, 0:64], src)
            nc.vector.tensor_copy(vv[:, :, 64], ones_col[:])

        def scores(qb, m, k):
            """Score pair (heads 2m,2m+1), sk-tile k, sq-block qb. The two
            K=64 matmuls use disjoint PE row groups + PSUM banks and stream
            concurrently. Returns the exp'd bf16 tile."""
            ss = ps_s.tile([128, 1024], F32, tag="ss", name="ss")
            for p2 in range(2):
                po = 64 * p2
                nc.tensor.matmul(
                    ss[:, p2 * 512:(p2 + 1) * 512],
                    kt_sb[m][po:po + 64, k * 128:(k + 1) * 128],
                    qt[m][po:po + 64, qb * 512:(qb + 1) * 512],
                    start=True, stop=True,
                )
            pt = pt_pool.tile([128, 1024], BF16, tag="pt", name="pt")
            nc.scalar.activation(
                pt[:], ss[:], AF.Exp, scale=1.0 / float(np.sqrt(HEAD_DIM)))
            return pt

        def pv(m, k, pt, accs):
            for p2 in range(2):
                h = 2 * m + p2
                nc.tensor.matmul(
                    accs[p2][:],
                    v_sb[k][:, h * 65:(h + 1) * 65],
                    pt[:, p2 * 512:(p2 + 1) * 512],
                    start=(k == 0), stop=(k == ST - 1),
                )

        def norm(qb, m, accs):
            """ot[m][:, qb block] = O^T / denom for both heads of pair m."""
            for p2 in range(2):
                recip_r = sm_pool.tile([1, 512], F32R, tag="recipr", name="recipr")
                with nc.allow_low_precision(reason="softmax denom"):
                    nc.vector.reciprocal(recip_r[:], accs[p2][64:65, :])
                rep = ps_w.tile([128, 512], F32, tag="pw", name="pw")
                nc.tensor.matmul(
                    rep[0:64, :], ones64_r[0:1, :], recip_r[0:1, :],
                    start=True, stop=True,
                )
                rep_sb = sm_pool.tile([64, 512], F32, tag="repsb", name="repsb")
                nc.vector.tensor_copy(rep_sb[:], rep[0:64, :])
                po = 64 * p2
                nc.vector.tensor_mul(
                    ot[m][po:po + 64, qb * 512:(qb + 1) * 512],
                    accs[p2][0:64, :], rep_sb[:])

        def yproj_i(i, ysb_holder):
            """Output projection for s-tile i; DMA when both halves done."""
            if ysb_holder[0] is None:
                ysb_holder[0] = y_pool.tile([128, D_MODEL], F32, tag="ysb", name="ysb")
            ysb = ysb_holder[0]
            for nb2 in range(2):
                ps = ps_w.tile([128, 512], F32, tag="pw", name="pw")
                for m in range(2):
                    nc.tensor.matmul(
                        ps[:],
                        ot[m][:, i * 128:(i + 1) * 128],
                        wo_r[m][:, nb2 * 512:(nb2 + 1) * 512],
                        start=(m == 0), stop=(m == 1),
                    )
                nc.vector.tensor_copy(
                    ysb[:, nb2 * 512:(nb2 + 1) * 512], ps[:])
            nc.sync.dma_start(y[i * 128:(i + 1) * 128, :], ysb[:])
            ysb_holder[0] = None

        # =============== emission schedule ===============================
        pairs = [(qb, m) for qb in range(SB) for m in range(2)]

        # lead-in: K block 0 (both halves), Q block 0
        for m in range(2):
            proj_qk_m("k", kt_sb, bk_c, xk_c, 0, m)
        for m in range(2):
            proj_qk_m("q", qt, bq_c, xq_c, 0, m)

        # p0: scores of pair (0,0); K blocks 1-3 + Q block 1 as PE filler
        pts_prev = []
        for k in range(ST):
            pts_prev.append(scores(0, 0, k))
            if k in (2, 4, 6):          # K blocks 1..3
                nb = k // 2
                for m in range(2):
                    proj_qk_m("k", kt_sb, bk_c, xk_c, nb, m)
            elif k in (9, 12):          # Q block 1
                proj_qk_m("q", qt, bq_c, xq_c, 1, 0 if k == 9 else 1)

        # windows p1..p7: scores of pair p run while the PREVIOUS pair's PV
        # drains (its pts are held; one pair of lag), then norm(prev).
        # p1 additionally weaves in the 16 V-projection chunks; later
        # windows weave in Q blocks 2-3 and the output projection.
        def fillers_for(p, k, yh):
            qb = pairs[p][0]
            if p in (2, 4) and k in (5, 11):      # Q blocks 2,3
                proj_qk_m("q", qt, bq_c, xq_c, qb + 1, 0 if k == 5 else 1)
            elif p in (3, 5, 7) and k % 4 == 2:   # yproj of qb-1 (4 s-tiles)
                yproj_i((qb - 1) * 4 + k // 4, yh)

        yh = [None]
        prev_pair = (0, 0)
        for p in range(1, len(pairs)):
            qb, m = pairs[p]
            accs_run = [ps_acc.tile([65, 512], F32, tag=f"acc{pp}",
                                    name=f"acc{pp}") for pp in range(2)]
            pts_cur = []
            for k in range(ST):
                pts_cur.append(scores(qb, m, k))
                if p == 1:
                    v_chunk(k)
                pv(prev_pair[1], k, pts_prev[k], accs_run)
                fillers_for(p, k, yh)
            norm(prev_pair[0], prev_pair[1], accs_run)
            prev_pair, pts_prev = (qb, m), pts_cur

        # tail window: PV + norm of the last pair, then yproj of block 3
        accs_run = [ps_acc.tile([65, 512], F32, tag=f"acc{pp}",
                                name=f"acc{pp}") for pp in range(2)]
        for k in range(ST):
            pv(prev_pair[1], k, pts_prev[k], accs_run)
        norm(prev_pair[0], prev_pair[1], accs_run)
        for i4 in range(4):
            yproj_i(3 * 4 + i4, yh)


def _get_nc():
    global _cached_nc
    if _cached_nc is None:
        _cached_nc = build_nc()
    return _cached_nc


def _make_in_maps(query, key, value, Wq, bq, Wk, bk, Wv, bv, Wo):
    """Shard + transpose + bf16-cast on host: core c = (b, hg), b = c // HG."""
    query = np.asarray(query, dtype=np.float32)
    key = np.asarray(key, dtype=np.float32)
    value = np.asarray(value, dtype=np.float32)
    Wq, Wk, Wv, Wo = (np.asarray(w, dtype=np.float32) for w in (Wq, Wk, Wv, Wo))
    bq, bk, bv = (np.asarray(b_, dtype=np.float32) for b_ in (bq, bk, bv))
    in_maps = []
    xq_t = [np.ascontiguousarray(query[b].T).astype(BF16_NP) for b in range(B)]
    xk_t = [np.ascontiguousarray(key[b].T).astype(BF16_NP) for b in range(B)]
    xv_t = [np.ascontiguousarray(value[b].T).astype(BF16_NP) for b in range(B)]
    for c in range(N_CORES):
        b, hg = divmod(c, HG)
        hs = slice(hg * DH, (hg + 1) * DH)
        in_maps.append({
            "xq_t": xq_t[b],
            "xk_t": xk_t[b],
            "xv_t": xv_t[b],
            "wq_t": np.ascontiguousarray(Wq[hs, :].T).astype(BF16_NP),
            "wk_t": np.ascontiguousarray(Wk[hs, :].T).astype(BF16_NP),
            "wv_t": np.ascontiguousarray(Wv[hs, :].T).astype(BF16_NP),
            "wo_t": np.ascontiguousarray(Wo[:, hs].T).astype(BF16_NP),
            "bqc": np.ascontiguousarray(bq[hs].reshape(2, 128)),
            "bkc": np.ascontiguousarray(bk[hs].reshape(2, 128)),
            "bv": np.ascontiguousarray(bv[hs]).reshape(1, DH).astype(BF16_NP),
        })
    return in_maps


def run(inputs, trace=False, **spmd_kwargs):
    nc = _get_nc()
    in_maps = _make_in_maps(
        inputs["query"], inputs["key"], inputs["value"],
        inputs["Wq"], inputs["bq"], inputs["Wk"], inputs["bk"],
        inputs["Wv"], inputs["bv"], inputs["Wo"])
    res = run_bass_kernel_spmd(
        nc, in_maps, list(range(N_CORES)), trace=trace, **spmd_kwargs)
    bo = np.asarray(inputs["bo"], dtype=np.float32)
    out = np.empty((B, S, D_MODEL), dtype=np.float32)
    for b in range(B):
        acc = np.zeros((S, D_MODEL), dtype=np.float32)
        for hg in range(HG):
            acc += res.results[b * HG + hg]["y"]
        out[b] = acc + bo
    return out, res


def kernel(**inputs) -> np.ndarray:
    out, _ = run(inputs, trace=False)
    return out


# revision 35
# speedup vs baseline: 1.3847x; 1.0386x over previous
"""MultiHeadAttention TRN2 Bass kernel (B=2, S=2048, D=1024, H=16, d=64).

Sharding: 8 cores = 2 (batch) x 4 (head groups of 4 heads).
Each core computes, for its batch b and head slice hs (256 dims):
    K^T = (Wk[hs,:] @ x_k^T + bk)    [256, 2048]   (dh on partitions)
    Q^T likewise; V = x_v @ Wv[hs,:].T + bv        [2048, 256]  (s on partitions)
    per head pair (2m, 2m+1): S^T = K_h @ Q_h^T, the two heads' score
    matmuls occupy disjoint PE row groups (contraction 64 at partitions
    0-63 / 64-127) and different PSUM banks -> they stream concurrently.
    P^T = exp(S^T / 8)   (scores ~ N(0,1), exp is safe without max-sub)
    [O^T ; denom] = [V_h | 1]^T @ P^T   (ones column folds the softmax
                                         denominator into the PV matmul)
    O^T = O^T * (1/denom)  (reciprocal_approx_fast, PE K=1 replicate)
    y_partial = O^T.T @ Wo[:, hs].T     [2048, 1024]
Host: y[b] = sum of 4 head-group partials + bo.

Everything the PE streams is bf16 (host-side cast: halves DMA, removes
all DVE casts, 1 cycle/row matmuls). The schedule is paced by the two
hard floors: Scalar-engine exp over 16.8M score elements (~137us) and
PE matmul rows (~110us). x is DMA'd in [128,512] column chunks through
rotating pools so the first score matmul lands ~10us in; after that the
emission keeps Scalar saturated: per sk-tile the PE emits scores(k)
BEFORE pv(k-1) (software pipeline, so the PE never blocks on the exp it
is feeding), and projection/output-projection work is woven into the
per-pair PE slack.
"""

import numpy as np
import ml_dtypes

import concourse.bass as bass
import concourse.tile as tile
import concourse.mybir as mybir
from concourse import bacc
from concourse.bass_utils import run_bass_kernel_spmd

D_MODEL = 1024
NUM_HEADS = 16
HEAD_DIM = 64
B, S = 2, 2048
N_CORES = 8
HG = 4                  # head-groups
HEADS_PER_CORE = NUM_HEADS // HG        # 4
DH = HEADS_PER_CORE * HEAD_DIM          # 256 output dims per core
KT = D_MODEL // 128                     # 8 contraction tiles
ST = S // 128                           # 16 sequence tiles
SB = S // 512                           # 4 sequence blocks of 512

F32 = mybir.dt.float32
F32R = mybir.dt.float32r
BF16 = mybir.dt.bfloat16
AF = mybir.ActivationFunctionType
BF16_NP = ml_dtypes.bfloat16

_cached_nc = None


def build_nc():
    nc = bacc.Bacc("TRN2", target_bir_lowering=False, debug=False)

    xq_t = nc.declare_dram_parameter("xq_t", [D_MODEL, S], BF16, isOutput=False)
    xk_t = nc.declare_dram_parameter("xk_t", [D_MODEL, S], BF16, isOutput=False)
    xv_t = nc.declare_dram_parameter("xv_t", [D_MODEL, S], BF16, isOutput=False)
    wq_t = nc.declare_dram_parameter("wq_t", [128, KT * DH], BF16, isOutput=False)
    wk_t = nc.declare_dram_parameter("wk_t", [128, KT * DH], BF16, isOutput=False)
    wv_t = nc.declare_dram_parameter("wv_t", [128, KT * DH], BF16, isOutput=False)
    wo_t = nc.declare_dram_parameter("wo_t", [128, 2 * D_MODEL], BF16, isOutput=False)
    bqk = nc.declare_dram_parameter("bqk", [128, 4], F32, isOutput=False)
    bv = nc.declare_dram_parameter("bv", [1, DH], BF16, isOutput=False)
    y = nc.declare_dram_parameter("y", [S, D_MODEL], F32, isOutput=True)

    with tile.TileContext(nc) as tc:
        _emit(nc, tc, xq_t, xk_t, xv_t, wq_t, wk_t, wv_t, wo_t, bqk, bv, y)
    nc.compile()
    return nc


def _emit(nc, tc, xq_t, xk_t, xv_t, wq_t, wk_t, wv_t, wo_t, bqk, bv, y):
    from contextlib import ExitStack

    ctx = ExitStack()
    with ctx:
        # ---- persistent tiles -------------------------------------------
        persist = ctx.enter_context(tc.tile_pool(name="persist", bufs=1))
        qt = [persist.tile([128, S], BF16, tag=f"qt{m}", name=f"qt{m}")
              for m in range(2)]
        kt_sb = [persist.tile([128, S], BF16, tag=f"kt{m}", name=f"kt{m}")
                 for m in range(2)]
        v_sb = [persist.tile([128, HEADS_PER_CORE * 65], BF16, tag=f"v{i}",
                             name=f"v{i}") for i in range(ST)]
        ot = [persist.tile([128, S], BF16, tag=f"ot{m}", name=f"ot{m}")
              for m in range(2)]
        wo_flat = persist.tile([128, 2 * D_MODEL], BF16, tag="wof", name="wof")
        wo_r = [wo_flat[:, m * D_MODEL:(m + 1) * D_MODEL] for m in range(2)]
        ones_row = persist.tile([1, S], BF16, tag="ones")
        ones64 = persist.tile([1, 64], F32, tag="ones64")
        ones64_r = persist.tile([1, 64], F32R, tag="ones64r")
        ones_col = persist.tile([128, HEADS_PER_CORE], F32, tag="onesc")
        bqk_c = persist.tile([128, 4], F32, tag="bqk")  # bq|bk per-partition
        bq_c, bk_c = bqk_c[:, 0:2], bqk_c[:, 2:4]
        bv_r = persist.tile([1, DH], BF16, tag="bvr")
        w_flat = {n: persist.tile([128, KT * DH], BF16, tag=f"w{n}",
                                  name=f"w{n}") for n in ("k", "q", "v")}
        w_sb = {n: [w_flat[n][:, k * DH:(k + 1) * DH] for k in range(KT)]
                for n in ("k", "q", "v")}

        # ---- x pools: fast-start [128,512] chunks + bulk remainder ------
        xk_pool = ctx.enter_context(tc.tile_pool(name="xk", bufs=8))
        xkb_pool = ctx.enter_context(tc.tile_pool(name="xkb", bufs=8))
        xq_pool = ctx.enter_context(tc.tile_pool(name="xq", bufs=16))
        xqb_pool = ctx.enter_context(tc.tile_pool(name="xqb", bufs=8))
        xv_pool = ctx.enter_context(tc.tile_pool(name="xv", bufs=8))
        xc_store = {}

        def dma_chunks(pool, key, dram, nb):
            for k in range(KT):
                t = pool.tile([128, 512], BF16, tag="xc", name="xc")
                nc.sync.dma_start(
                    t[:], dram[k * 128:(k + 1) * 128,
                               nb * 512:(nb + 1) * 512])
                xc_store[(key, k, nb)] = (t, 0)

        def dma_bulk(pool, key, dram, nb0, nbn, tag):
            w = (nbn - nb0) * 512
            for k in range(KT):
                t = pool.tile([128, w], BF16, tag=tag, name="xb")
                nc.sync.dma_start(
                    t[:], dram[k * 128:(k + 1) * 128,
                               nb0 * 512:nbn * 512])
                for nb in range(nb0, nbn):
                    xc_store[(key, k, nb)] = (t, (nb - nb0) * 512)

        def xs(key, k, nb):
            t, off = xc_store[(key, k, nb)]
            return t[:, off:off + 512]

        # DMA priority order == consumption order
        nc.sync.dma_start(bqk_c[:], bqk[:, :])
        nc.sync.dma_start(w_flat["k"][:], wk_t[:, :])
        nc.sync.dma_start(w_flat["q"][:], wq_t[:, :])
        dma_chunks(xk_pool, "k", xk_t, 0)
        dma_chunks(xq_pool, "q", xq_t, 0)
        dma_bulk(xkb_pool, "k", xk_t, 1, 4, "xkb")
        dma_chunks(xq_pool, "q", xq_t, 1)
        nc.sync.dma_start(w_flat["v"][:], wv_t[:, :])
        nc.sync.dma_start(bv_r[:], bv[:])
        dma_bulk(xv_pool, "v", xv_t, 0, 4, "xvb")
        dma_bulk(xqb_pool, "q", xq_t, 2, 4, "xqb")
        nc.sync.dma_start(wo_flat[:], wo_t[:, :])

        # ---- pipelined-body pools ---------------------------------------
        ps_s = ctx.enter_context(
            tc.tile_pool(name="pss", bufs=2, space="PSUM"))      # 4 banks
        ps_acc = ctx.enter_context(
            tc.tile_pool(name="psacc", bufs=1, space="PSUM"))    # 2 banks
        ps_w = ctx.enter_context(
            tc.tile_pool(name="psw", bufs=2, space="PSUM"))      # 2 banks
        pt_pool = ctx.enter_context(tc.tile_pool(name="pt", bufs=19))
        sm_pool = ctx.enter_context(tc.tile_pool(name="small", bufs=2))
        y_pool = ctx.enter_context(tc.tile_pool(name="ysb", bufs=2))

        # constants
        nc.gpsimd.memset(ones_row[:], 1.0)
        nc.vector.memset(ones64[:], 1.0)
        nc.vector.tensor_copy(ones64_r[:], ones64[:])
        nc.vector.memset(ones_col[:], 1.0)

        # ---- building blocks --------------------------------------------
        def proj_qk_m(name, dst, bias_c, nb, m):
            """Project one (512-col, m-half) block of Q^T or K^T."""
            ps = ps_w.tile([128, 512], F32, tag="pw", name="pw")
            for k in range(KT):
                nc.tensor.matmul(
                    ps[:],
                    w_sb[name][k][:, m * 128:(m + 1) * 128],
                    xs(name, k, nb),
                    start=(k == 0), stop=(k == KT - 1),
                )
            nc.scalar.activation(
                dst[m][:, nb * 512:(nb + 1) * 512], ps[:],
                AF.Identity, bias=bias_c[:, m:m + 1])

        def v_chunk(i):
            """Project V for s-tile i into v_sb[i] (+ ones column). One
            accumulation group per PSUM tile: interleaved groups sharing a
            bank clobber each other's has_written state."""
            nb, col = divmod(i, 4)
            ps = ps_w.tile([128, 512], F32, tag="pw", name="pw")
            for k in range(KT):
                nc.tensor.matmul(
                    ps[:, 0:256],
                    xs("v", k, nb)[:, col * 128:(col + 1) * 128],
                    w_sb["v"][k][:],
                    start=(k == 0), stop=False,
                )
            nc.tensor.matmul(
                ps[:, 0:256],
                ones_row[0:1, i * 128:(i + 1) * 128],
                bv_r[0:1, :],
                start=False, stop=True,
            )
            src = ps[:, 0:256].rearrange("p (h c) -> p h c", c=64)
            vv = v_sb[i].rearrange("p (h c) -> p h c", c=65)
            nc.vector.tensor_copy(vv[:, :, 0:64], src)
            nc.vector.tensor_copy(vv[:, :, 64], ones_col[:])

        def scores(qb, m, k):
            """Score pair (heads 2m,2m+1), sk-tile k, sq-block qb. The two
            K=64 matmuls use disjoint PE row groups + PSUM banks and stream
            concurrently. Returns the exp'd bf16 tile."""
            ss = ps_s.tile([128, 1024], F32, tag="ss", name="ss")
            for p2 in range(2):
                po = 64 * p2
                nc.tensor.matmul(
                    ss[:, p2 * 512:(p2 + 1) * 512],
                    kt_sb[m][po:po + 64, k * 128:(k + 1) * 128],
                    qt[m][po:po + 64, qb * 512:(qb + 1) * 512],
                    start=True, stop=True,
                )
            pt = pt_pool.tile([128, 1024], BF16, tag="pt", name="pt")
            nc.scalar.activation(
                pt[:], ss[:], AF.Exp, scale=1.0 / float(np.sqrt(HEAD_DIM)))
            return pt

        def pv(m, k, pt, accs):
            for p2 in range(2):
                h = 2 * m + p2
                nc.tensor.matmul(
                    accs[p2][:],
                    v_sb[k][:, h * 65:(h + 1) * 65],
                    pt[:, p2 * 512:(p2 + 1) * 512],
                    start=(k == 0), stop=(k == ST - 1),
                )

        def norm_recips(accs):
            """1/denom on DVE (approx-fast; ~51 ULP is plenty for softmax)."""
            out = []
            for p2 in range(2):
                recip_r = sm_pool.tile([1, 512], F32R, tag="recipr",
                                       name="recipr")
                with nc.allow_low_precision(reason="softmax denom"):
                    nc.vector.reciprocal(recip_r[:], accs[p2][64:65, :])
                out.append(recip_r)
            return out

        def norm_apply(qb, m, accs, recips):
            """ot[m][:, qb block] = O^T * recip (PE replicate + DVE mult)."""
            for p2 in range(2):
                rep = ps_w.tile([128, 512], F32, tag="pw", name="pw")
                nc.tensor.matmul(
                    rep[0:64, :], ones64_r[0:1, :], recips[p2][0:1, :],
                    start=True, stop=True,
                )
                rep_sb = sm_pool.tile([64, 512], F32, tag="repsb", name="repsb")
                nc.vector.tensor_copy(rep_sb[:], rep[0:64, :])
                po = 64 * p2
                nc.vector.tensor_mul(
                    ot[m][po:po + 64, qb * 512:(qb + 1) * 512],
                    accs[p2][0:64, :], rep_sb[:])

        def yproj_i(i, ysb_holder):
            """Output projection for s-tile i; DMA when both halves done."""
            if ysb_holder[0] is None:
                ysb_holder[0] = y_pool.tile([128, D_MODEL], F32, tag="ysb", name="ysb")
            ysb = ysb_holder[0]
            for nb2 in range(2):
                ps = ps_w.tile([128, 512], F32, tag="pw", name="pw")
                for m in range(2):
                    nc.tensor.matmul(
                        ps[:],
                        ot[m][:, i * 128:(i + 1) * 128],
                        wo_r[m][:, nb2 * 512:(nb2 + 1) * 512],
                        start=(m == 0), stop=(m == 1),
                    )
                nc.vector.tensor_copy(
                    ysb[:, nb2 * 512:(nb2 + 1) * 512], ps[:])
            nc.sync.dma_start(y[i * 128:(i + 1) * 128, :], ysb[:])
            ysb_holder[0] = None

        # =============== emission schedule ===============================
        pairs = [(qb, m) for qb in range(SB) for m in range(2)]

        # lead-in: K block 0 (both halves), Q block 0
        for m in range(2):
            proj_qk_m("k", kt_sb, bk_c, 0, m)
        for m in range(2):
            proj_qk_m("q", qt, bq_c, 0, m)

        # p0: scores of pair (0,0); K blocks 1-3 + Q block 1 as PE filler
        pts_prev = []
        for k in range(ST):
            pts_prev.append(scores(0, 0, k))
            if k in (2, 4, 6):          # K blocks 1..3
                nb = k // 2
                for m in range(2):
                    proj_qk_m("k", kt_sb, bk_c, nb, m)
            elif k in (9, 12):          # Q block 1
                proj_qk_m("q", qt, bq_c, 1, 0 if k == 9 else 1)

        # windows p1..p7: scores of pair p run while the PREVIOUS pair's PV
        # drains (its pts are held; one pair of lag), then norm(prev).
        # p1 additionally weaves in the 16 V-projection chunks; later
        # windows weave in Q blocks 2-3 and the output projection.
        def fillers_for(p, k, yh):
            qb = pairs[p][0]
            if p in (2, 4) and k in (5, 11):      # Q blocks 2,3
                proj_qk_m("q", qt, bq_c, qb + 1, 0 if k == 5 else 1)
            elif p in (3, 5, 7) and k % 4 == 2:   # yproj of qb-1 (4 s-tiles)
                yproj_i((qb - 1) * 4 + k // 4, yh)

        yh = [None]
        prev_pair = (0, 0)
        pending_norm = None      # (qb, m, accs, recips) awaiting norm_apply
        for p in range(1, len(pairs)):
            qb, m = pairs[p]
            accs_run = [ps_acc.tile([65, 512], F32, tag=f"acc{pp}",
                                    name=f"acc{pp}") for pp in range(2)]
            pts_cur = []
            for k in range(ST):
                pts_cur.append(scores(qb, m, k))
                if k == 0 and pending_norm is not None:
                    norm_apply(*pending_norm)
                    pending_norm = None
                if p == 1:
                    v_chunk(k)
                pv(prev_pair[1], k, pts_prev[k], accs_run)
                fillers_for(p, k, yh)
            recips = norm_recips(accs_run)
            pending_norm = (prev_pair[0], prev_pair[1], accs_run, recips)
            prev_pair, pts_prev = (qb, m), pts_cur

        # tail window: PV + norm of the last pair, then yproj of block 3
        accs_run = [ps_acc.tile([65, 512], F32, tag=f"acc{pp}",
                                name=f"acc{pp}") for pp in range(2)]
        for k in range(ST):
            if k == 0 and pending_norm is not None:
                norm_apply(*pending_norm)
                pending_norm = None
            pv(prev_pair[1], k, pts_prev[k], accs_run)
        recips = norm_recips(accs_run)
        norm_apply(prev_pair[0], prev_pair[1], accs_run, recips)
        for i4 in range(4):
            yproj_i(3 * 4 + i4, yh)


def _get_nc():
    global _cached_nc
    if _cached_nc is None:
        _cached_nc = build_nc()
    return _cached_nc


def _make_in_maps(query, key, value, Wq, bq, Wk, bk, Wv, bv, Wo):
    """Shard + transpose + bf16-cast on host: core c = (b, hg), b = c // HG."""
    query = np.asarray(query, dtype=np.float32)
    key = np.asarray(key, dtype=np.float32)
    value = np.asarray(value, dtype=np.float32)
    Wq, Wk, Wv, Wo = (np.asarray(w, dtype=np.float32) for w in (Wq, Wk, Wv, Wo))
    bq, bk, bv = (np.asarray(b_, dtype=np.float32) for b_ in (bq, bk, bv))
    in_maps = []
    xq_t = [np.ascontiguousarray(query[b].T).astype(BF16_NP) for b in range(B)]
    xk_t = [np.ascontiguousarray(key[b].T).astype(BF16_NP) for b in range(B)]
    xv_t = [np.ascontiguousarray(value[b].T).astype(BF16_NP) for b in range(B)]
    def tile_w(WT):          # [1024, 256] -> [128, 8*256] (k-tiles packed)
        return np.ascontiguousarray(
            WT.reshape(KT, 128, DH).transpose(1, 0, 2).reshape(128, KT * DH)
        ).astype(BF16_NP)

    for c in range(N_CORES):
        b, hg = divmod(c, HG)
        hs = slice(hg * DH, (hg + 1) * DH)
        wo_tiled = np.ascontiguousarray(
            Wo[:, hs].T.reshape(2, 128, D_MODEL).transpose(1, 0, 2)
            .reshape(128, 2 * D_MODEL)).astype(BF16_NP)
        bqk_pack = np.concatenate(
            [bq[hs].reshape(2, 128).T, bk[hs].reshape(2, 128).T],
            axis=1)          # [128, 4] = bq cols | bk cols
        in_maps.append({
            "xq_t": xq_t[b],
            "xk_t": xk_t[b],
            "xv_t": xv_t[b],
            "wq_t": tile_w(Wq[hs, :].T),
            "wk_t": tile_w(Wk[hs, :].T),
            "wv_t": tile_w(Wv[hs, :].T),
            "wo_t": wo_tiled,
            "bqk": np.ascontiguousarray(bqk_pack),
            "bv": np.ascontiguousarray(bv[hs]).reshape(1, DH).astype(BF16_NP),
        })
    return in_maps


def run(inputs, trace=False, **spmd_kwargs):
    nc = _get_nc()
    in_maps = _make_in_maps(
        inputs["query"], inputs["key"], inputs["value"],
        inputs["Wq"], inputs["bq"], inputs["Wk"], inputs["bk"],
        inputs["Wv"], inputs["bv"], inputs["Wo"])
    res = run_bass_kernel_spmd(
        nc, in_maps, list(range(N_CORES)), trace=trace, **spmd_kwargs)
    bo = np.asarray(inputs["bo"], dtype=np.float32)
    out = np.empty((B, S, D_MODEL), dtype=np.float32)
    for b in range(B):
        acc = np.zeros((S, D_MODEL), dtype=np.float32)
        for hg in range(HG):
            acc += res.results[b * HG + hg]["y"]
        out[b] = acc + bo
    return out, res


def kernel(**inputs) -> np.ndarray:
    out, _ = run(inputs, trace=False)
    return out


# revision 40
# speedup vs baseline: 1.4209x; 1.0261x over previous
"""MultiHeadAttention TRN2 Bass kernel (B=2, S=2048, D=1024, H=16, d=64).

Sharding: 8 cores = 2 (batch) x 4 (head groups of 4 heads).
Each core computes, for its batch b and head slice hs (256 dims):
    K^T = (Wk[hs,:] @ x_k^T + bk)    [256, 2048]   (dh on partitions)
    Q^T likewise; V = x_v @ Wv[hs,:].T + bv        [2048, 256]  (s on partitions)
    per head pair (2m, 2m+1): S^T = K_h @ Q_h^T, the two heads' score
    matmuls occupy disjoint PE row groups (contraction 64 at partitions
    0-63 / 64-127) and different PSUM banks -> they stream concurrently.
    P^T = exp(S^T / 8)   (scores ~ N(0,1), exp is safe without max-sub)
    [O^T ; denom] = [V_h | 1]^T @ P^T   (ones column folds the softmax
                                         denominator into the PV matmul)
    O^T = O^T * (1/denom)  (reciprocal_approx_fast, PE K=1 replicate)
    y_partial = O^T.T @ Wo[:, hs].T     [2048, 1024]
Host: y[b] = sum of 4 head-group partials + bo.

Everything the PE streams is bf16 (host-side cast: halves DMA, removes
all DVE casts, 1 cycle/row matmuls). The schedule is paced by the two
hard floors: Scalar-engine exp over 16.8M score elements (~137us) and
PE matmul rows (~110us). x is DMA'd in [128,512] column chunks through
rotating pools so the first score matmul lands ~10us in; after that the
emission keeps Scalar saturated: per sk-tile the PE emits scores(k)
BEFORE pv(k-1) (software pipeline, so the PE never blocks on the exp it
is feeding), and projection/output-projection work is woven into the
per-pair PE slack.
"""

import numpy as np
import ml_dtypes

import concourse.bass as bass
import concourse.tile as tile
import concourse.mybir as mybir
from concourse import bacc
from concourse.bass_utils import run_bass_kernel_spmd

D_MODEL = 1024
NUM_HEADS = 16
HEAD_DIM = 64
B, S = 2, 2048
N_CORES = 8
HG = 4                  # head-groups
HEADS_PER_CORE = NUM_HEADS // HG        # 4
DH = HEADS_PER_CORE * HEAD_DIM          # 256 output dims per core
KT = D_MODEL // 128                     # 8 contraction tiles
ST = S // 128                           # 16 sequence tiles
SB = S // 512                           # 4 sequence blocks of 512

F32 = mybir.dt.float32
F32R = mybir.dt.float32r
BF16 = mybir.dt.bfloat16
AF = mybir.ActivationFunctionType
BF16_NP = ml_dtypes.bfloat16

_cached_nc = None


def build_nc():
    nc = bacc.Bacc("TRN2", target_bir_lowering=False, debug=False)

    xq_t = nc.declare_dram_parameter("xq_t", [D_MODEL, S], BF16, isOutput=False)
    xk_t = nc.declare_dram_parameter("xk_t", [D_MODEL, S], BF16, isOutput=False)
    xv_t = nc.declare_dram_parameter("xv_t", [D_MODEL, S], BF16, isOutput=False)
    wq_t = nc.declare_dram_parameter("wq_t", [128, KT * DH], BF16, isOutput=False)
    wk_t = nc.declare_dram_parameter("wk_t", [128, KT * DH], BF16, isOutput=False)
    wv_t = nc.declare_dram_parameter("wv_t", [128, KT * DH], BF16, isOutput=False)
    wo_t = nc.declare_dram_parameter("wo_t", [128, 2 * D_MODEL], BF16, isOutput=False)
    bqk = nc.declare_dram_parameter("bqk", [128, 4], F32, isOutput=False)
    bv = nc.declare_dram_parameter("bv", [1, DH], BF16, isOutput=False)
    y = nc.declare_dram_parameter("y", [S, D_MODEL], F32, isOutput=True)

    with tile.TileContext(nc) as tc:
        _emit(nc, tc, xq_t, xk_t, xv_t, wq_t, wk_t, wv_t, wo_t, bqk, bv, y)
    nc.compile()
    return nc


def _emit(nc, tc, xq_t, xk_t, xv_t, wq_t, wk_t, wv_t, wo_t, bqk, bv, y):
    from contextlib import ExitStack

    ctx = ExitStack()
    with ctx:
        # ---- persistent tiles -------------------------------------------
        persist = ctx.enter_context(tc.tile_pool(name="persist", bufs=1))
        qt = [persist.tile([128, S], BF16, tag=f"qt{m}", name=f"qt{m}")
              for m in range(2)]
        kt_sb = [persist.tile([128, S], BF16, tag=f"kt{m}", name=f"kt{m}")
                 for m in range(2)]
        v_sb = [persist.tile([128, HEADS_PER_CORE * 65], BF16, tag=f"v{i}",
                             name=f"v{i}") for i in range(ST)]
        ot = [persist.tile([128, S], BF16, tag=f"ot{m}", name=f"ot{m}")
              for m in range(2)]
        wo_flat = persist.tile([128, 2 * D_MODEL], BF16, tag="wof", name="wof")
        wo_r = [wo_flat[:, m * D_MODEL:(m + 1) * D_MODEL] for m in range(2)]
        ones_row = persist.tile([1, S], BF16, tag="ones")
        ones64 = persist.tile([33, 64], F32, tag="ones64")
        ones64_r = persist.tile([33, 64], F32R, tag="ones64r")
        ones_col = persist.tile([128, HEADS_PER_CORE], F32, tag="onesc")
        bqk_c = persist.tile([128, 4], F32, tag="bqk")  # bq|bk per-partition
        bq_c, bk_c = bqk_c[:, 0:2], bqk_c[:, 2:4]
        bv_r = persist.tile([1, DH], BF16, tag="bvr")
        w_flat = {n: persist.tile([128, KT * DH], BF16, tag=f"w{n}",
                                  name=f"w{n}") for n in ("k", "q", "v")}
        w_sb = {n: [w_flat[n][:, k * DH:(k + 1) * DH] for k in range(KT)]
                for n in ("k", "q", "v")}

        # ---- x pools: fast-start [128,512] chunks + bulk remainder ------
        xk_pool = ctx.enter_context(tc.tile_pool(name="xk", bufs=8))
        xkb_pool = ctx.enter_context(tc.tile_pool(name="xkb", bufs=8))
        xq_pool = ctx.enter_context(tc.tile_pool(name="xq", bufs=8))
        xqb_pool = ctx.enter_context(tc.tile_pool(name="xqb", bufs=8))
        xv_pool = ctx.enter_context(tc.tile_pool(name="xv", bufs=8))
        xc_store = {}

        def dma_chunks(pool, key, dram, nb):
            for k in range(KT):
                t = pool.tile([128, 512], BF16, tag="xc", name="xc")
                nc.sync.dma_start(
                    t[:], dram[k * 128:(k + 1) * 128,
                               nb * 512:(nb + 1) * 512])
                xc_store[(key, k, nb)] = (t, 0)

        def dma_bulk(pool, key, dram, nb0, nbn, tag):
            w = (nbn - nb0) * 512
            for k in range(KT):
                t = pool.tile([128, w], BF16, tag=tag, name="xb")
                nc.sync.dma_start(
                    t[:], dram[k * 128:(k + 1) * 128,
                               nb0 * 512:nbn * 512])
                for nb in range(nb0, nbn):
                    xc_store[(key, k, nb)] = (t, (nb - nb0) * 512)

        def xs(key, k, nb):
            t, off = xc_store[(key, k, nb)]
            return t[:, off:off + 512]

        # DMA priority order == consumption order
        nc.sync.dma_start(bqk_c[:], bqk[:, :])
        nc.sync.dma_start(w_flat["k"][:], wk_t[:, :])
        nc.sync.dma_start(w_flat["q"][:], wq_t[:, :])
        dma_chunks(xk_pool, "k", xk_t, 0)
        dma_chunks(xq_pool, "q", xq_t, 0)
        dma_bulk(xkb_pool, "k", xk_t, 1, 4, "xkb")
        dma_bulk(xqb_pool, "q", xq_t, 1, 4, "xqb")
        nc.sync.dma_start(w_flat["v"][:], wv_t[:, :])
        nc.sync.dma_start(bv_r[:], bv[:])
        dma_bulk(xv_pool, "v", xv_t, 0, 4, "xvb")
        nc.sync.dma_start(wo_flat[:], wo_t[:, :])

        # ---- pipelined-body pools ---------------------------------------
        ps_s = ctx.enter_context(
            tc.tile_pool(name="pss", bufs=2, space="PSUM"))      # 4 banks
        ps_acc = ctx.enter_context(
            tc.tile_pool(name="psacc", bufs=1, space="PSUM"))    # 2 banks
        ps_w = ctx.enter_context(
            tc.tile_pool(name="psw", bufs=2, space="PSUM"))      # 2 banks
        pt_pool = ctx.enter_context(tc.tile_pool(name="pt", bufs=19))
        sm_pool = ctx.enter_context(tc.tile_pool(name="small", bufs=1))
        sm2_pool = ctx.enter_context(tc.tile_pool(name="small2", bufs=2))
        y_pool = ctx.enter_context(tc.tile_pool(name="ysb", bufs=2))

        # constants
        nc.gpsimd.memset(ones_row[:], 1.0)
        nc.vector.memset(ones64[:], 1.0)
        nc.vector.tensor_copy(ones64_r[:], ones64[:])
        nc.vector.memset(ones_col[:], 1.0)

        # ---- building blocks --------------------------------------------
        def proj_qk_m(name, dst, bias_c, nb, m):
            """Project one (512-col, m-half) block of Q^T or K^T."""
            ps = ps_w.tile([128, 512], F32, tag="pw", name="pw")
            for k in range(KT):
                nc.tensor.matmul(
                    ps[:],
                    w_sb[name][k][:, m * 128:(m + 1) * 128],
                    xs(name, k, nb),
                    start=(k == 0), stop=(k == KT - 1),
                )
            nc.scalar.activation(
                dst[m][:, nb * 512:(nb + 1) * 512], ps[:],
                AF.Identity, bias=bias_c[:, m:m + 1])

        def v_chunk(i):
            """Project V for s-tile i into v_sb[i] (+ ones column). One
            accumulation group per PSUM tile: interleaved groups sharing a
            bank clobber each other's has_written state."""
            nb, col = divmod(i, 4)
            ps = ps_w.tile([128, 512], F32, tag="pw", name="pw")
            for k in range(KT):
                nc.tensor.matmul(
                    ps[:, 0:256],
                    xs("v", k, nb)[:, col * 128:(col + 1) * 128],
                    w_sb["v"][k][:],
                    start=(k == 0), stop=False,
                )
            nc.tensor.matmul(
                ps[:, 0:256],
                ones_row[0:1, i * 128:(i + 1) * 128],
                bv_r[0:1, :],
                start=False, stop=True,
            )
            src = ps[:, 0:256].rearrange("p (h c) -> p h c", c=64)
            vv = v_sb[i].rearrange("p (h c) -> p h c", c=65)
            nc.vector.tensor_copy(vv[:, :, 0:64], src)
            nc.vector.tensor_copy(vv[:, :, 64], ones_col[:])

        def scores(qb, m, k):
            """Score pair (heads 2m,2m+1), sk-tile k, sq-block qb. The two
            K=64 matmuls use disjoint PE row groups + PSUM banks and stream
            concurrently. Returns the exp'd bf16 tile."""
            ss = ps_s.tile([128, 1024], F32, tag="ss", name="ss")
            for p2 in range(2):
                po = 64 * p2
                nc.tensor.matmul(
                    ss[:, p2 * 512:(p2 + 1) * 512],
                    kt_sb[m][po:po + 64, k * 128:(k + 1) * 128],
                    qt[m][po:po + 64, qb * 512:(qb + 1) * 512],
                    start=True, stop=True,
                )
            pt = pt_pool.tile([128, 1024], BF16, tag="pt", name="pt")
            nc.scalar.activation(
                pt[:], ss[:], AF.Exp, scale=1.0 / float(np.sqrt(HEAD_DIM)))
            return pt

        def pv(m, k, pt, accs):
            for p2 in range(2):
                h = 2 * m + p2
                nc.tensor.matmul(
                    accs[p2][:],
                    v_sb[k][:, h * 65:(h + 1) * 65],
                    pt[:, p2 * 512:(p2 + 1) * 512],
                    start=(k == 0), stop=(k == ST - 1),
                )

        def norm_stage1(accs):
            """Evict O rows + denominators to SBUF (frees the PSUM accs for
            the next pair immediately) and start the batched reciprocal."""
            o_sb = []
            den2 = sm_pool.tile([33, 512], F32, tag="den2", name="den2")
            for p2 in range(2):
                o = sm2_pool.tile([64, 512], BF16, tag=f"o{p2}", name="osb")
                nc.vector.tensor_copy(o[:], accs[p2][0:64, :])
                o_sb.append(o)
                nc.vector.tensor_copy(den2[32 * p2:32 * p2 + 1, :],
                                      accs[p2][64:65, :])
            recip2 = sm2_pool.tile([33, 512], F32R, tag="recip2", name="recip2")
            with nc.allow_low_precision(reason="softmax denom"):
                nc.vector.reciprocal(recip2[0:1, :], den2[0:1, :])
                nc.vector.reciprocal(recip2[32:33, :], den2[32:33, :])
            return (o_sb, recip2)

        def norm_apply(qb, m, st):
            """ot[m][:, qb block] = O^T * recip: PE K=1 replicate + GpSimd
            multiply (SBUF-only operands, keeps DVE free)."""
            o_sb, recip2 = st
            for p2 in range(2):
                rep = ps_w.tile([128, 512], F32, tag="pw", name="pw")
                nc.tensor.matmul(
                    rep[0:64, :], ones64_r[32 * p2:32 * p2 + 1, :],
                    recip2[32 * p2:32 * p2 + 1, :],
                    start=True, stop=True,
                )
                rep_sb = sm_pool.tile([64, 512], BF16, tag="repsb",
                                      name="repsb")
                nc.vector.tensor_copy(rep_sb[:], rep[0:64, :])
                po = 64 * p2
                nc.gpsimd.tensor_mul(
                    ot[m][po:po + 64, qb * 512:(qb + 1) * 512],
                    o_sb[p2][:], rep_sb[:])

        def yproj_i(i, ysb_holder):
            """Output projection for s-tile i; DMA when both halves done."""
            if ysb_holder[0] is None:
                ysb_holder[0] = y_pool.tile([128, D_MODEL], F32, tag="ysb", name="ysb")
            ysb = ysb_holder[0]
            for nb2 in range(2):
                ps = ps_w.tile([128, 512], F32, tag="pw", name="pw")
                for m in range(2):
                    nc.tensor.matmul(
                        ps[:],
                        ot[m][:, i * 128:(i + 1) * 128],
                        wo_r[m][:, nb2 * 512:(nb2 + 1) * 512],
                        start=(m == 0), stop=(m == 1),
                    )
                nc.vector.tensor_copy(
                    ysb[:, nb2 * 512:(nb2 + 1) * 512], ps[:])
            nc.sync.dma_start(y[i * 128:(i + 1) * 128, :], ysb[:])
            ysb_holder[0] = None

        # =============== emission schedule ===============================
        pairs = [(qb, m) for qb in range(SB) for m in range(2)]

        # lead-in: K block 0 (both halves), Q block 0
        for m in range(2):
            proj_qk_m("k", kt_sb, bk_c, 0, m)
        for m in range(2):
            proj_qk_m("q", qt, bq_c, 0, m)

        # p0: scores of pair (0,0); K blocks 1-3 + Q block 1 as PE filler
        pts_prev = []
        for k in range(ST):
            pts_prev.append(scores(0, 0, k))
            if k in (2, 4, 6):          # K blocks 1..3
                nb = k // 2
                for m in range(2):
                    proj_qk_m("k", kt_sb, bk_c, nb, m)
            elif k in (9, 12):          # Q block 1
                proj_qk_m("q", qt, bq_c, 1, 0 if k == 9 else 1)

        # windows p1..p7: scores of pair p run while the PREVIOUS pair's PV
        # drains (its pts are held; one pair of lag), then norm(prev).
        # p1 additionally weaves in the 16 V-projection chunks; later
        # windows weave in Q blocks 2-3 and the output projection.
        def fillers_for(p, k, yh):
            qb = pairs[p][0]
            if p in (2, 4) and k in (5, 11):      # Q blocks 2,3
                proj_qk_m("q", qt, bq_c, qb + 1, 0 if k == 5 else 1)
            elif p in (3, 5, 7) and k >= 6 and k % 3 == 0:
                # yproj of qb-1 (4 s-tiles at k=6,9,12,15, after norm_apply)
                yproj_i((qb - 1) * 4 + (k - 6) // 3, yh)

        yh = [None]
        prev_pair = (0, 0)
        pending_norm = None      # (qb, m, accs, recips) awaiting norm_apply
        for p in range(1, len(pairs)):
            qb, m = pairs[p]
            accs_run = [ps_acc.tile([65, 512], F32, tag=f"acc{pp}",
                                    name=f"acc{pp}") for pp in range(2)]
            pts_cur = []
            for k in range(ST):
                pts_cur.append(scores(qb, m, k))
                if k == 5 and pending_norm is not None:
                    norm_apply(*pending_norm)
                    pending_norm = None
                if p == 1:
                    v_chunk(k)
                pv(prev_pair[1], k, pts_prev[k], accs_run)
                fillers_for(p, k, yh)
            st = norm_stage1(accs_run)
            pending_norm = (prev_pair[0], prev_pair[1], st)
            prev_pair, pts_prev = (qb, m), pts_cur

        # tail window: PV + norm of the last pair, then yproj of block 3
        accs_run = [ps_acc.tile([65, 512], F32, tag=f"acc{pp}",
                                name=f"acc{pp}") for pp in range(2)]
        for k in range(ST):
            if k == 5 and pending_norm is not None:
                norm_apply(*pending_norm)
                pending_norm = None
            pv(prev_pair[1], k, pts_prev[k], accs_run)
        st = norm_stage1(accs_run)
        norm_apply(prev_pair[0], prev_pair[1], st)
        for i4 in range(4):
            yproj_i(3 * 4 + i4, yh)


def _get_nc():
    global _cached_nc
    if _cached_nc is None:
        _cached_nc = build_nc()
    return _cached_nc


def _make_in_maps(query, key, value, Wq, bq, Wk, bk, Wv, bv, Wo):
    """Shard + transpose + bf16-cast on host: core c = (b, hg), b = c // HG."""
    query = np.asarray(query, dtype=np.float32)
    key = np.asarray(key, dtype=np.float32)
    value = np.asarray(value, dtype=np.float32)
    Wq, Wk, Wv, Wo = (np.asarray(w, dtype=np.float32) for w in (Wq, Wk, Wv, Wo))
    bq, bk, bv = (np.asarray(b_, dtype=np.float32) for b_ in (bq, bk, bv))
    in_maps = []
    xq_t = [np.ascontiguousarray(query[b].T).astype(BF16_NP) for b in range(B)]
    xk_t = [np.ascontiguousarray(key[b].T).astype(BF16_NP) for b in range(B)]
    xv_t = [np.ascontiguousarray(value[b].T).astype(BF16_NP) for b in range(B)]
    def tile_w(WT):          # [1024, 256] -> [128, 8*256] (k-tiles packed)
        return np.ascontiguousarray(
            WT.reshape(KT, 128, DH).transpose(1, 0, 2).reshape(128, KT * DH)
        ).astype(BF16_NP)

    for c in range(N_CORES):
        b, hg = divmod(c, HG)
        hs = slice(hg * DH, (hg + 1) * DH)
        wo_tiled = np.ascontiguousarray(
            Wo[:, hs].T.reshape(2, 128, D_MODEL).transpose(1, 0, 2)
            .reshape(128, 2 * D_MODEL)).astype(BF16_NP)
        bqk_pack = np.concatenate(
            [bq[hs].reshape(2, 128).T, bk[hs].reshape(2, 128).T],
            axis=1)          # [128, 4] = bq cols | bk cols
        in_maps.append({
            "xq_t": xq_t[b],
            "xk_t": xk_t[b],
            "xv_t": xv_t[b],
            "wq_t": tile_w(Wq[hs, :].T),
            "wk_t": tile_w(Wk[hs, :].T),
            "wv_t": tile_w(Wv[hs, :].T),
            "wo_t": wo_tiled,
            "bqk": np.ascontiguousarray(bqk_pack),
            "bv": np.ascontiguousarray(bv[hs]).reshape(1, DH).astype(BF16_NP),
        })
    return in_maps


def run(inputs, trace=False, **spmd_kwargs):
    nc = _get_nc()
    in_maps = _make_in_maps(
        inputs["query"], inputs["key"], inputs["value"],
        inputs["Wq"], inputs["bq"], inputs["Wk"], inputs["bk"],
        inputs["Wv"], inputs["bv"], inputs["Wo"])
    res = run_bass_kernel_spmd(
        nc, in_maps, list(range(N_CORES)), trace=trace, **spmd_kwargs)
    bo = np.asarray(inputs["bo"], dtype=np.float32)
    out = np.empty((B, S, D_MODEL), dtype=np.float32)
    for b in range(B):
        acc = np.zeros((S, D_MODEL), dtype=np.float32)
        for hg in range(HG):
            acc += res.results[b * HG + hg]["y"]
        out[b] = acc + bo
    return out, res


def kernel(**inputs) -> np.ndarray:
    out, _ = run(inputs, trace=False)
    return out


# revision 41
# speedup vs baseline: 1.6108x; 1.1337x over previous
"""MultiHeadAttention TRN2 Bass kernel (B=2, S=2048, D=1024, H=16, d=64).

Sharding: 8 cores = 2 (batch) x 4 (head groups of 4 heads).
Each core computes, for its batch b and head slice hs (256 dims):
    K^T = (Wk[hs,:] @ x_k^T + bk)    [256, 2048]   (dh on partitions)
    Q^T likewise; V = x_v @ Wv[hs,:].T + bv        [2048, 256]  (s on partitions)
    per head pair (2m, 2m+1): S^T = K_h @ Q_h^T, the two heads' score
    matmuls occupy disjoint PE row groups (contraction 64 at partitions
    0-63 / 64-127) and different PSUM banks -> they stream concurrently.
    P^T = exp(S^T / 8)   (scores ~ N(0,1), exp is safe without max-sub)
    [O^T ; denom] = [V_h | 1]^T @ P^T   (ones column folds the softmax
                                         denominator into the PV matmul)
    O^T = O^T * (1/denom)  (reciprocal_approx_fast, PE K=1 replicate)
    y_partial = O^T.T @ Wo[:, hs].T     [2048, 1024]
Host: y[b] = sum of 4 head-group partials + bo.

Everything the PE streams is bf16 (host-side cast: halves DMA, removes
all DVE casts, 1 cycle/row matmuls). The schedule is paced by the two
hard floors: Scalar-engine exp over 16.8M score elements (~137us) and
PE matmul rows (~110us). x is DMA'd in [128,512] column chunks through
rotating pools so the first score matmul lands ~10us in; after that the
emission keeps Scalar saturated: per sk-tile the PE emits scores(k)
BEFORE pv(k-1) (software pipeline, so the PE never blocks on the exp it
is feeding), and projection/output-projection work is woven into the
per-pair PE slack.
"""

import numpy as np
import ml_dtypes

import concourse.bass as bass
import concourse.tile as tile
import concourse.mybir as mybir
from concourse import bacc
from concourse.bass_utils import run_bass_kernel_spmd

D_MODEL = 1024
NUM_HEADS = 16
HEAD_DIM = 64
B, S = 2, 2048
N_CORES = 8
HG = 4                  # head-groups
HEADS_PER_CORE = NUM_HEADS // HG        # 4
DH = HEADS_PER_CORE * HEAD_DIM          # 256 output dims per core
KT = D_MODEL // 128                     # 8 contraction tiles
ST = S // 128                           # 16 sequence tiles
SB = S // 512                           # 4 sequence blocks of 512

F32 = mybir.dt.float32
F32R = mybir.dt.float32r
BF16 = mybir.dt.bfloat16
AF = mybir.ActivationFunctionType
BF16_NP = ml_dtypes.bfloat16

_cached_nc = None


def build_nc():
    nc = bacc.Bacc("TRN2", target_bir_lowering=False, debug=False)

    xq_t = nc.declare_dram_parameter("xq_t", [D_MODEL, S], BF16, isOutput=False)
    xk_t = nc.declare_dram_parameter("xk_t", [D_MODEL, S], BF16, isOutput=False)
    xv_t = nc.declare_dram_parameter("xv_t", [D_MODEL, S], BF16, isOutput=False)
    wq_t = nc.declare_dram_parameter("wq_t", [128, KT * DH], BF16, isOutput=False)
    wk_t = nc.declare_dram_parameter("wk_t", [128, KT * DH], BF16, isOutput=False)
    wv_t = nc.declare_dram_parameter("wv_t", [128, KT * DH], BF16, isOutput=False)
    wo_t = nc.declare_dram_parameter("wo_t", [128, 2 * D_MODEL], BF16, isOutput=False)
    bqk = nc.declare_dram_parameter("bqk", [128, 4], F32, isOutput=False)
    bv = nc.declare_dram_parameter("bv", [1, DH], BF16, isOutput=False)
    y = nc.declare_dram_parameter("y", [S, D_MODEL], F32, isOutput=True)

    with tile.TileContext(nc) as tc:
        _emit(nc, tc, xq_t, xk_t, xv_t, wq_t, wk_t, wv_t, wo_t, bqk, bv, y)
    nc.compile()
    return nc


def _emit(nc, tc, xq_t, xk_t, xv_t, wq_t, wk_t, wv_t, wo_t, bqk, bv, y):
    from contextlib import ExitStack

    ctx = ExitStack()
    with ctx:
        # ---- persistent tiles -------------------------------------------
        persist = ctx.enter_context(tc.tile_pool(name="persist", bufs=1))
        qt = [persist.tile([128, S], BF16, tag=f"qt{m}", name=f"qt{m}")
              for m in range(2)]
        kt_sb = [persist.tile([128, S], BF16, tag=f"kt{m}", name=f"kt{m}")
                 for m in range(2)]
        v_sb = [persist.tile([128, HEADS_PER_CORE * 65], BF16, tag=f"v{i}",
                             name=f"v{i}") for i in range(ST)]
        ot = [persist.tile([128, S], BF16, tag=f"ot{m}", name=f"ot{m}")
              for m in range(2)]
        wo_flat = persist.tile([128, 2 * D_MODEL], BF16, tag="wof", name="wof")
        wo_r = [wo_flat[:, m * D_MODEL:(m + 1) * D_MODEL] for m in range(2)]
        ones_row = persist.tile([1, S], BF16, tag="ones")
        ones64 = persist.tile([33, 64], F32, tag="ones64")
        ones64_r = persist.tile([33, 64], F32R, tag="ones64r")
        ones_col = persist.tile([128, HEADS_PER_CORE], F32, tag="onesc")
        bqk_c = persist.tile([128, 4], F32, tag="bqk")  # bq|bk per-partition
        bq_c, bk_c = bqk_c[:, 0:2], bqk_c[:, 2:4]
        bv_r = persist.tile([1, DH], BF16, tag="bvr")
        w_flat = {n: persist.tile([128, KT * DH], BF16, tag=f"w{n}",
                                  name=f"w{n}") for n in ("k", "q", "v")}
        w_sb = {n: [w_flat[n][:, k * DH:(k + 1) * DH] for k in range(KT)]
                for n in ("k", "q", "v")}

        # ---- x pools: fast-start [128,512] chunks + bulk remainder ------
        xk_pool = ctx.enter_context(tc.tile_pool(name="xk", bufs=8))
        xkb_pool = ctx.enter_context(tc.tile_pool(name="xkb", bufs=8))
        xq_pool = ctx.enter_context(tc.tile_pool(name="xq", bufs=8))
        xqb_pool = ctx.enter_context(tc.tile_pool(name="xqb", bufs=8))
        xv_pool = ctx.enter_context(tc.tile_pool(name="xv", bufs=8))
        xc_store = {}

        def dma_chunks(pool, key, dram, nb):
            for k in range(KT):
                t = pool.tile([128, 512], BF16, tag="xc", name="xc")
                nc.sync.dma_start(
                    t[:], dram[k * 128:(k + 1) * 128,
                               nb * 512:(nb + 1) * 512])
                xc_store[(key, k, nb)] = (t, 0)

        def dma_bulk(pool, key, dram, nb0, nbn, tag):
            w = (nbn - nb0) * 512
            for k in range(KT):
                t = pool.tile([128, w], BF16, tag=tag, name="xb")
                nc.sync.dma_start(
                    t[:], dram[k * 128:(k + 1) * 128,
                               nb0 * 512:nbn * 512])
                for nb in range(nb0, nbn):
                    xc_store[(key, k, nb)] = (t, (nb - nb0) * 512)

        def xs(key, k, nb):
            t, off = xc_store[(key, k, nb)]
            return t[:, off:off + 512]

        # DMA priority order == consumption order
        nc.sync.dma_start(bqk_c[:], bqk[:, :])
        nc.sync.dma_start(w_flat["k"][:], wk_t[:, :])
        nc.sync.dma_start(w_flat["q"][:], wq_t[:, :])
        dma_chunks(xk_pool, "k", xk_t, 0)
        dma_chunks(xq_pool, "q", xq_t, 0)
        dma_bulk(xkb_pool, "k", xk_t, 1, 4, "xkb")
        dma_bulk(xqb_pool, "q", xq_t, 1, 4, "xqb")
        nc.sync.dma_start(w_flat["v"][:], wv_t[:, :])
        nc.sync.dma_start(bv_r[:], bv[:])
        dma_bulk(xv_pool, "v", xv_t, 0, 4, "xvb")
        nc.sync.dma_start(wo_flat[:], wo_t[:, :])

        # ---- pipelined-body pools ---------------------------------------
        ps_s = ctx.enter_context(
            tc.tile_pool(name="pss", bufs=2, space="PSUM"))      # 4 banks
        ps_acc = ctx.enter_context(
            tc.tile_pool(name="psacc", bufs=1, space="PSUM"))    # 2 banks
        ps_w = ctx.enter_context(
            tc.tile_pool(name="psw", bufs=2, space="PSUM"))      # 2 banks
        pt_pool = ctx.enter_context(tc.tile_pool(name="pt", bufs=19))
        sm_pool = ctx.enter_context(tc.tile_pool(name="small", bufs=1))
        sm2_pool = ctx.enter_context(tc.tile_pool(name="small2", bufs=2))
        y_pool = ctx.enter_context(tc.tile_pool(name="ysb", bufs=2))

        # constants
        nc.gpsimd.memset(ones_row[:], 1.0)
        nc.vector.memset(ones64[:], 1.0)
        nc.vector.tensor_copy(ones64_r[:], ones64[:])
        nc.vector.memset(ones_col[:], 1.0)

        # ---- building blocks --------------------------------------------
        def proj_qk_m(name, dst, bias_c, nb, m):
            """Project one (512-col, m-half) block of Q^T or K^T."""
            ps = ps_w.tile([128, 512], F32, tag="pw", name="pw")
            for k in range(KT):
                nc.tensor.matmul(
                    ps[:],
                    w_sb[name][k][:, m * 128:(m + 1) * 128],
                    xs(name, k, nb),
                    start=(k == 0), stop=(k == KT - 1),
                )
            nc.scalar.activation(
                dst[m][:, nb * 512:(nb + 1) * 512], ps[:],
                AF.Identity, bias=bias_c[:, m:m + 1])

        def v_chunk(i):
            """Project V for s-tile i into v_sb[i] (+ ones column). One
            accumulation group per PSUM tile: interleaved groups sharing a
            bank clobber each other's has_written state."""
            nb, col = divmod(i, 4)
            ps = ps_w.tile([128, 512], F32, tag="pw", name="pw")
            for k in range(KT):
                nc.tensor.matmul(
                    ps[:, 0:256],
                    xs("v", k, nb)[:, col * 128:(col + 1) * 128],
                    w_sb["v"][k][:],
                    start=(k == 0), stop=False,
                )
            nc.tensor.matmul(
                ps[:, 0:256],
                ones_row[0:1, i * 128:(i + 1) * 128],
                bv_r[0:1, :],
                start=False, stop=True,
            )
            src = ps[:, 0:256].rearrange("p (h c) -> p h c", c=64)
            vv = v_sb[i].rearrange("p (h c) -> p h c", c=65)
            nc.vector.tensor_copy(vv[:, :, 0:64], src)
            nc.vector.tensor_copy(vv[:, :, 64], ones_col[:])

        def scores(qb, m, k):
            """Score pair (heads 2m,2m+1), sk-tile k, sq-block qb. The two
            K=64 matmuls use disjoint PE row groups + PSUM banks and stream
            concurrently. Returns the exp'd bf16 tile."""
            ss = ps_s.tile([128, 1024], F32, tag="ss", name="ss")
            for p2 in range(2):
                po = 64 * p2
                nc.tensor.matmul(
                    ss[:, p2 * 512:(p2 + 1) * 512],
                    kt_sb[m][po:po + 64, k * 128:(k + 1) * 128],
                    qt[m][po:po + 64, qb * 512:(qb + 1) * 512],
                    start=True, stop=True,
                )
            pt = pt_pool.tile([128, 1024], BF16, tag="pt", name="pt")
            nc.scalar.activation(
                pt[:], ss[:], AF.Exp, scale=1.0 / float(np.sqrt(HEAD_DIM)))
            return pt

        def pv(m, k, pt, accs):
            for p2 in range(2):
                h = 2 * m + p2
                nc.tensor.matmul(
                    accs[p2][:],
                    v_sb[k][:, h * 65:(h + 1) * 65],
                    pt[:, p2 * 512:(p2 + 1) * 512],
                    start=(k == 0), stop=(k == ST - 1),
                )

        def norm_stage1(accs):
            """Evict O rows + denominators to SBUF (frees the PSUM accs for
            the next pair immediately) and start the batched reciprocal."""
            o_sb = []
            den2 = sm_pool.tile([33, 512], F32, tag="den2", name="den2")
            for p2 in range(2):
                o = sm2_pool.tile([64, 512], BF16, tag=f"o{p2}", name="osb")
                nc.vector.tensor_copy(o[:], accs[p2][0:64, :])
                o_sb.append(o)
                nc.vector.tensor_copy(den2[32 * p2:32 * p2 + 1, :],
                                      accs[p2][64:65, :])
            recip2 = sm2_pool.tile([33, 512], F32R, tag="recip2", name="recip2")
            with nc.allow_low_precision(reason="softmax denom"):
                nc.vector.reciprocal(recip2[:], den2[:])
            return (o_sb, recip2)

        def norm_apply(qb, m, st):
            """ot[m][:, qb block] = O^T * recip: PE K=1 replicate + GpSimd
            multiply (SBUF-only operands, keeps DVE free)."""
            o_sb, recip2 = st
            for p2 in range(2):
                rep = ps_w.tile([128, 512], F32, tag="pw", name="pw")
                nc.tensor.matmul(
                    rep[0:64, :], ones64_r[32 * p2:32 * p2 + 1, :],
                    recip2[32 * p2:32 * p2 + 1, :],
                    start=True, stop=True,
                )
                rep_sb = sm_pool.tile([64, 512], BF16, tag="repsb",
                                      name="repsb")
                nc.vector.tensor_copy(rep_sb[:], rep[0:64, :])
                po = 64 * p2
                nc.gpsimd.tensor_mul(
                    ot[m][po:po + 64, qb * 512:(qb + 1) * 512],
                    o_sb[p2][:], rep_sb[:])

        def yproj_i(i, ysb_holder):
            """Output projection for s-tile i; DMA when both halves done."""
            if ysb_holder[0] is None:
                ysb_holder[0] = y_pool.tile([128, D_MODEL], F32, tag="ysb", name="ysb")
            ysb = ysb_holder[0]
            for nb2 in range(2):
                ps = ps_w.tile([128, 512], F32, tag="pw", name="pw")
                for m in range(2):
                    nc.tensor.matmul(
                        ps[:],
                        ot[m][:, i * 128:(i + 1) * 128],
                        wo_r[m][:, nb2 * 512:(nb2 + 1) * 512],
                        start=(m == 0), stop=(m == 1),
                    )
                nc.vector.tensor_copy(
                    ysb[:, nb2 * 512:(nb2 + 1) * 512], ps[:])
            nc.sync.dma_start(y[i * 128:(i + 1) * 128, :], ysb[:])
            ysb_holder[0] = None

        # =============== emission schedule ===============================
        pairs = [(qb, m) for qb in range(SB) for m in range(2)]

        # lead-in: K block 0 (both halves), Q block 0
        for m in range(2):
            proj_qk_m("k", kt_sb, bk_c, 0, m)
        for m in range(2):
            proj_qk_m("q", qt, bq_c, 0, m)

        # p0: scores of pair (0,0); K blocks 1-3 + Q block 1 as PE filler
        pts_prev = []
        for k in range(ST):
            pts_prev.append(scores(0, 0, k))
            if k in (2, 4, 6):          # K blocks 1..3
                nb = k // 2
                for m in range(2):
                    proj_qk_m("k", kt_sb, bk_c, nb, m)
            elif k in (9, 12):          # Q block 1
                proj_qk_m("q", qt, bq_c, 1, 0 if k == 9 else 1)

        # windows p1..p7: scores of pair p run while the PREVIOUS pair's PV
        # drains (its pts are held; one pair of lag), then norm(prev).
        # p1 additionally weaves in the 16 V-projection chunks; later
        # windows weave in Q blocks 2-3 and the output projection.
        def fillers_for(p, k, yh):
            qb = pairs[p][0]
            if p in (2, 4) and k in (5, 11):      # Q blocks 2,3
                proj_qk_m("q", qt, bq_c, qb + 1, 0 if k == 5 else 1)
            elif p in (3, 5, 7) and k >= 6 and k % 3 == 0:
                # yproj of qb-1 (4 s-tiles at k=6,9,12,15, after norm_apply)
                yproj_i((qb - 1) * 4 + (k - 6) // 3, yh)

        yh = [None]
        prev_pair = (0, 0)
        pending_norm = None      # (qb, m, accs, recips) awaiting norm_apply
        for p in range(1, len(pairs)):
            qb, m = pairs[p]
            accs_run = [ps_acc.tile([65, 512], F32, tag=f"acc{pp}",
                                    name=f"acc{pp}") for pp in range(2)]
            pts_cur = []
            for k in range(ST):
                pts_cur.append(scores(qb, m, k))
                if k == 5 and pending_norm is not None:
                    norm_apply(*pending_norm)
                    pending_norm = None
                if p == 1:
                    v_chunk(k)
                pv(prev_pair[1], k, pts_prev[k], accs_run)
                fillers_for(p, k, yh)
            st = norm_stage1(accs_run)
            pending_norm = (prev_pair[0], prev_pair[1], st)
            prev_pair, pts_prev = (qb, m), pts_cur

        # tail window: PV + norm of the last pair, then yproj of block 3
        accs_run = [ps_acc.tile([65, 512], F32, tag=f"acc{pp}",
                                name=f"acc{pp}") for pp in range(2)]
        for k in range(ST):
            if k == 5 and pending_norm is not None:
                norm_apply(*pending_norm)
                pending_norm = None
            pv(prev_pair[1], k, pts_prev[k], accs_run)
        st = norm_stage1(accs_run)
        norm_apply(prev_pair[0], prev_pair[1], st)
        for i4 in range(4):
            yproj_i(3 * 4 + i4, yh)


def _get_nc():
    global _cached_nc
    if _cached_nc is None:
        _cached_nc = build_nc()
    return _cached_nc


def _make_in_maps(query, key, value, Wq, bq, Wk, bk, Wv, bv, Wo):
    """Shard + transpose + bf16-cast on host: core c = (b, hg), b = c // HG."""
    query = np.asarray(query, dtype=np.float32)
    key = np.asarray(key, dtype=np.float32)
    value = np.asarray(value, dtype=np.float32)
    Wq, Wk, Wv, Wo = (np.asarray(w, dtype=np.float32) for w in (Wq, Wk, Wv, Wo))
    bq, bk, bv = (np.asarray(b_, dtype=np.float32) for b_ in (bq, bk, bv))
    in_maps = []
    xq_t = [np.ascontiguousarray(query[b].T).astype(BF16_NP) for b in range(B)]
    xk_t = [np.ascontiguousarray(key[b].T).astype(BF16_NP) for b in range(B)]
    xv_t = [np.ascontiguousarray(value[b].T).astype(BF16_NP) for b in range(B)]
    def tile_w(WT):          # [1024, 256] -> [128, 8*256] (k-tiles packed)
        return np.ascontiguousarray(
            WT.reshape(KT, 128, DH).transpose(1, 0, 2).reshape(128, KT * DH)
        ).astype(BF16_NP)

    for c in range(N_CORES):
        b, hg = divmod(c, HG)
        hs = slice(hg * DH, (hg + 1) * DH)
        wo_tiled = np.ascontiguousarray(
            Wo[:, hs].T.reshape(2, 128, D_MODEL).transpose(1, 0, 2)
            .reshape(128, 2 * D_MODEL)).astype(BF16_NP)
        bqk_pack = np.concatenate(
            [bq[hs].reshape(2, 128).T, bk[hs].reshape(2, 128).T],
            axis=1)          # [128, 4] = bq cols | bk cols
        in_maps.append({
            "xq_t": xq_t[b],
            "xk_t": xk_t[b],
            "xv_t": xv_t[b],
            "wq_t": tile_w(Wq[hs, :].T),
            "wk_t": tile_w(Wk[hs, :].T),
            "wv_t": tile_w(Wv[hs, :].T),
            "wo_t": wo_tiled,
            "bqk": np.ascontiguousarray(bqk_pack),
            "bv": np.ascontiguousarray(bv[hs]).reshape(1, DH).astype(BF16_NP),
        })
    return in_maps


def run(inputs, trace=False, **spmd_kwargs):
    nc = _get_nc()
    in_maps = _make_in_maps(
        inputs["query"], inputs["key"], inputs["value"],
        inputs["Wq"], inputs["bq"], inputs["Wk"], inputs["bk"],
        inputs["Wv"], inputs["bv"], inputs["Wo"])
    res = run_bass_kernel_spmd(
        nc, in_maps, list(range(N_CORES)), trace=trace, **spmd_kwargs)
    bo = np.asarray(inputs["bo"], dtype=np.float32)
    out = np.empty((B, S, D_MODEL), dtype=np.float32)
    for b in range(B):
        acc = np.zeros((S, D_MODEL), dtype=np.float32)
        for hg in range(HG):
            acc += res.results[b * HG + hg]["y"]
        out[b] = acc + bo
    return out, res


def kernel(**inputs) -> np.ndarray:
    out, _ = run(inputs, trace=False)
    return out
